# revision 1
# baseline (speedup 1.0000x reference)
"""Trainium2 Bass kernel for NearestNeighborSparseLayer.

Reference computation:
    eff = connections * nearest_neighbors * weight.T   # [in, out]
    out = x @ eff + bias                                # [8192, 4096]

`nearest_neighbors` is a tridiagonal mask (|i-j| <= 1), so `eff` has at
most 3 nonzero diagonals and the matmul collapses to a banded (3-tap)
elementwise operation along the feature axis:

    out[t, j] = x[t, j-1]*cA[j] + x[t, j]*cB[j] + x[t, j+1]*cC[j] + bias[j]

where cA[j] = eff[j-1, j], cB[j] = eff[j, j], cC[j] = eff[j+1, j].

Strategy: data-parallel over the 8192 token rows across 8 NeuronCores
(1024 rows/core).  The host only slices/reformats data (sharding, band
gathering via np.diagonal, replication); all arithmetic — the
connections*nearest_neighbors*weight products and the banded multiply-
accumulate — runs on-device.

If `nearest_neighbors` is NOT band-limited (never the case for this
problem's input generator, which builds a tridiagonal mask), we fall
back to a plain numpy evaluation for correctness.
"""

import os

import numpy as np

BATCH = 8192
FEAT = 4096
N_CORES = 8
TOK_PER_CORE = BATCH // N_CORES  # 1024
P = 128  # partitions

LAST_RESULTS = None  # BassKernelResults from the most recent run (for test.py)

_cached = {}  # (has_bias,) -> compiled Bass program


def _build_banded_program(has_bias: bool):
    import concourse.bass as bass  # noqa: F401
    import concourse.mybir as mybir
    import concourse.tile as tile
    from concourse import bacc

    f32 = mybir.dt.float32
    mult = mybir.AluOpType.mult
    add = mybir.AluOpType.add

    nc = bacc.Bacc("TRN2", target_bir_lowering=False, debug=False)

    x_d = nc.dram_tensor("x", [TOK_PER_CORE, FEAT], f32, kind="ExternalInput").ap()
    cb_d = nc.dram_tensor("conn_band", [3, FEAT], f32, kind="ExternalInput").ap()
    nb_d = nc.dram_tensor("nn_band", [3, FEAT], f32, kind="ExternalInput").ap()
    wb_d = nc.dram_tensor("w_band", [3, FEAT], f32, kind="ExternalInput").ap()
    if has_bias:
        bias_d = nc.dram_tensor("bias", [1, FEAT], f32, kind="ExternalInput").ap()
    y_d = nc.dram_tensor("y", [TOK_PER_CORE, FEAT], f32, kind="ExternalOutput").ap()

    n_tiles = TOK_PER_CORE // P  # 8

    # bands live as [96, 128] tiles (3*4096 elements spread over 96
    # partitions) so they cost 512B/partition instead of 16KB/partition
    bp, bf = 96, 128

    with tile.TileContext(nc) as tc:
        with (
            tc.tile_pool(name="const", bufs=1) as const,
            tc.tile_pool(name="xp", bufs=2) as xp,
            tc.tile_pool(name="tp", bufs=2) as tp,
            tc.tile_pool(name="dram", bufs=1, space="DRAM") as dram,
        ):
            # --- one-time: compute banded coefficients on device ---
            cb_sb = const.tile([bp, bf], f32, tag="cb")
            nb_sb = const.tile([bp, bf], f32, tag="nb")
            wb_sb = const.tile([bp, bf], f32, tag="wb")
            r96 = lambda ap: ap.rearrange("a (b c) -> (a b) c", c=bf)
            nc.sync.dma_start(out=cb_sb[:], in_=r96(cb_d))
            nc.sync.dma_start(out=nb_sb[:], in_=r96(nb_d))
            nc.sync.dma_start(out=wb_sb[:], in_=r96(wb_d))
            coef = const.tile([bp, bf], f32, tag="coef")
            nc.vector.tensor_tensor(coef[:], cb_sb[:], nb_sb[:], mult)
            nc.vector.tensor_tensor(coef[:], coef[:], wb_sb[:], mult)

            # round-trip through DRAM so we can broadcast each row across
            # all 128 partitions with a step-0 DMA read
            coef_dram = dram.tile([3, FEAT], f32, tag="coefd")
            nc.sync.dma_start(out=r96(coef_dram[:]), in_=coef[:])

            A = const.tile([P, FEAT], f32, tag="A")
            B = const.tile([P, FEAT], f32, tag="B")
            C = const.tile([P, FEAT], f32, tag="C")
            nc.sync.dma_start(out=A[:], in_=coef_dram[0:1, :].broadcast_to([P, FEAT]))
            nc.sync.dma_start(out=B[:], in_=coef_dram[1:2, :].broadcast_to([P, FEAT]))
            nc.sync.dma_start(out=C[:], in_=coef_dram[2:3, :].broadcast_to([P, FEAT]))
            if has_bias:
                BI = const.tile([P, FEAT], f32, tag="BI")
                nc.sync.dma_start(
                    out=BI[:], in_=bias_d[0:1, :].broadcast_to([P, FEAT])
                )

            # --- main loop: banded 3-tap multiply-accumulate ---
            for i in range(n_tiles):
                r0 = i * P
                xt = xp.tile([P, FEAT + 2], f32, tag="x")
                nc.vector.memset(xt[:, 0:1], 0.0)
                nc.vector.memset(xt[:, FEAT + 1 : FEAT + 2], 0.0)
                nc.sync.dma_start(out=xt[:, 1 : FEAT + 1], in_=x_d[r0 : r0 + P, :])

                t_a = tp.tile([P, FEAT], f32, tag="ta")
                t_b = tp.tile([P, FEAT], f32, tag="tb")
                t_c = tp.tile([P, FEAT], f32, tag="tc")

                # x[t, j-1] * cA[j]
                nc.vector.tensor_tensor(t_a[:], xt[:, 0:FEAT], A[:], mult)
                # x[t, j+1] * cC[j]
                nc.vector.tensor_tensor(t_c[:], xt[:, 2 : FEAT + 2], C[:], mult)
                # x[t, j] * cB[j]   (gpsimd runs in parallel with DVE)
                nc.gpsimd.tensor_tensor(t_b[:], xt[:, 1 : FEAT + 1], B[:], mult)
                # t_a += t_c  (in-place: identical in/out APs are safe for
                # elementwise streaming ops)
                nc.vector.tensor_tensor(t_a[:], t_a[:], t_c[:], add)
                if has_bias:
                    nc.gpsimd.tensor_tensor(t_b[:], t_b[:], BI[:], add)
                nc.gpsimd.tensor_tensor(t_b[:], t_a[:], t_b[:], add)

                nc.sync.dma_start(out=y_d[r0 : r0 + P, :], in_=t_b[:])

    nc.compile()
    return nc


def _pe_chunks():
    """Non-overlapping column chunks for the PE-banded kernel.

    Chunk c produces output columns [C_c, C_c + N_c) from input rows
    [R_c, R_c + K_c), where the 3-diagonal band makes each column depend on
    rows col-1..col+1.  With R_c = 126*c the row windows fit in 128
    partitions and every output column is produced by exactly ONE matmul
    (no PSUM accumulation).  delta = C_c - R_c selects which diagonals of
    the rhs block are populated.

    Returns list of (c, R, K, C, N, delta).
    """
    chunks = []
    c = 0
    col = 0
    while col < FEAT:
        R = 126 * c
        K = min(P, FEAT - R)
        delta = col - R  # 0 for chunk 0, 1 afterwards
        max_col = FEAT - 1 if R + K >= FEAT else R + K - 2
        N = max_col - col + 1
        chunks.append((c, R, K, col, N, delta))
        col += N
        c += 1
    return chunks


def _build_banded_pe_program(has_bias: bool):
    """v2: banded matmul on the tensor engine, non-overlapping chunks.

    For each chunk (R, K, C, N, delta):
        out[tokens, C:C+N] = xT[R:R+K, tokens].T @ E_c[0:K, 0:N]
    where E_c is the dense banded block of eff rows R..R+K-1 x cols
    C..C+N-1, built on device from the gathered diagonals.  Every output
    column is produced by exactly one matmul (start=stop=True), so no
    PSUM accumulation semantics are needed.
    """
    import concourse.bass as bass  # noqa: F401
    import concourse.mybir as mybir
    import concourse.tile as tile
    from concourse import bacc

    f32 = mybir.dt.float32
    mult = mybir.AluOpType.mult
    add = mybir.AluOpType.add

    nc = bacc.Bacc("TRN2", target_bir_lowering=False, debug=False)

    chunks = _pe_chunks()
    n_chunks = len(chunks)  # 33
    n_m = TOK_PER_CORE // P  # 8
    NB = n_chunks  # band columns per diagonal

    xT_d = nc.dram_tensor("xT", [FEAT, TOK_PER_CORE], f32, kind="ExternalInput").ap()
    # bands packed [128, 3*NB]: col d*NB + c holds band_d[126c + p] at
    # partition p (d: 0=u sub, 1=v main, 2=w super diag of eff's rows)
    cb_d = nc.dram_tensor("cbT", [P, 3 * NB], f32, kind="ExternalInput").ap()
    nb_d = nc.dram_tensor("nbT", [P, 3 * NB], f32, kind="ExternalInput").ap()
    wb_d = nc.dram_tensor("wbT", [P, 3 * NB], f32, kind="ExternalInput").ap()
    if has_bias:
        bias_d = nc.dram_tensor("bias", [1, FEAT], f32, kind="ExternalInput").ap()
    y_d = nc.dram_tensor("y", [TOK_PER_CORE, FEAT], f32, kind="ExternalOutput").ap()

    with tile.TileContext(nc) as tc:
        with (
            tc.tile_pool(name="const", bufs=1) as const,
            tc.tile_pool(name="xp", bufs=1) as xp,
            tc.tile_pool(name="op", bufs=int(os.environ.get("KERNEL_OPBUFS", "2"))) as op,
            tc.tile_pool(name="pp", bufs=8, space="PSUM") as pp,
        ):
            # IDW[p, q] = 1 iff p == q-1; slicing IDW[:, d+1 : d+1+N] gives
            # the shifted identity J_d[p, q] = [p == q+d] for d in -1..2
            idw = const.tile([P, P + 2], f32, tag="idw")
            nc.gpsimd.memset(idw[:], 0.0)
            nc.gpsimd.affine_select(
                out=idw[:],
                in_=idw[:],
                compare_op=mybir.AluOpType.not_equal,
                fill=1.0,
                base=1,
                # fill where (p - q + 1) == 0, i.e. at q = p+1
                pattern=[[-1, P + 2]],
                channel_multiplier=1,
            )

            cb_sb = const.tile([P, 3 * NB], f32, tag="cb")
            nb_sb = const.tile([P, 3 * NB], f32, tag="nb")
            wb_sb = const.tile([P, 3 * NB], f32, tag="wb")
            nc.sync.dma_start(out=cb_sb[:], in_=cb_d[:])
            nc.sync.dma_start(out=nb_sb[:], in_=nb_d[:])
            nc.sync.dma_start(out=wb_sb[:], in_=wb_d[:])
            uvw = const.tile([P, 3 * NB], f32, tag="uvw")
            nc.vector.tensor_tensor(uvw[:], cb_sb[:], nb_sb[:], mult)
            nc.vector.tensor_tensor(uvw[:], uvw[:], wb_sb[:], mult)

            if has_bias:
                bias_bc = const.tile([P, FEAT], f32, tag="biasbc")
                nc.sync.dma_start(
                    out=bias_bc[:], in_=bias_d[0:1, :].broadcast_to([P, FEAT])
                )

            def jd(d, n):  # shifted identity J_d [128, n]
                return idw[:, d + 1 : d + 1 + n]

            def sv(d, c):  # per-partition band scalar for diag d, chunk c
                return uvw[:, d * NB + c : d * NB + c + 1]

            # E_c[p, q] = eff[R+p, C+q]: diag d=p-q==delta-1 -> w[R+p],
            # ==delta -> v[R+p], ==delta+1 -> u[R+p]
            eblocks = []
            for c, R, K, C, N, delta in chunks:
                E = const.tile([P, P + 1], f32, tag=f"E{c}", name=f"E{c}")
                nc.vector.tensor_scalar(
                    E[:, 0:N], jd(delta - 1, N), sv(2, c), None, mult
                )
                nc.vector.scalar_tensor_tensor(
                    E[:, 0:N], jd(delta, N), sv(1, c), E[:, 0:N], mult, add
                )
                nc.vector.scalar_tensor_tensor(
                    E[:, 0:N], jd(delta + 1, N), sv(0, c), E[:, 0:N], mult, add
                )
                eblocks.append(E)

            # whole xT shard in SBUF once, as 33 overlapping row-slabs
            # [K, 1024] (~132KB/partition); reused by all 8 m-blocks
            X = xp.tile([P, n_chunks, TOK_PER_CORE], f32, tag="X")
            for c, R, K, C, N, delta in chunks:
                nc.sync.dma_start(out=X[0:K, c, :], in_=xT_d[R : R + K, :])

            ablate = os.environ.get("KERNEL_ABLATE", "")
            # chunks grouped 4-per-PSUM-bank: the first matmul in a group
            # arms the 2KB bank (start=True); later matmuls overwrite their
            # own still-pending columns; one copy evicts the whole group.
            GRP = int(os.environ.get("KERNEL_GRP", "1"))
            groups = [chunks[i : i + GRP] for i in range(0, n_chunks, GRP)]
            # out DMA piece boundaries, in units of groups
            per = int(os.environ.get("KERNEL_PIECE_GROUPS", "0")) or max(1, len(chunks) // (4 * GRP))
            cmode = os.environ.get("KERNEL_COPY", "a")
            for m in range(n_m):
                t0 = m * P
                out_m = op.tile([P, FEAT], f32, tag="out")
                if ablate:
                    nc.vector.memset(out_m[:, 0:1], 0.0)
                col0 = 0
                for g, grp in enumerate(groups):
                    gC = grp[0][3]  # first col of group
                    gH = grp[-1][3] + grp[-1][4]  # end col
                    if "nomm" not in ablate:
                        pt = pp.tile([P, 512], f32, tag="ps", name=f"ps_{m}_{g}")
                        for j, (c, R, K, C, N, delta) in enumerate(grp):
                            nc.tensor.matmul(
                                pt[0:P, C - gC : C - gC + N],
                                X[0:K, c, t0 : t0 + P],
                                eblocks[c][0:K, 0:N],
                                start=(j == 0),
                                stop=(j == len(grp) - 1),
                            )
                        if "nocopy" not in ablate:
                            eng = [ch for ch in cmode][g % len(cmode)]
                            if eng == "v":
                                nc.vector.tensor_copy(
                                    out_m[:, gC:gH], pt[:, 0 : gH - gC]
                                )
                            elif eng == "s":
                                nc.scalar.copy(
                                    out_m[:, gC:gH], pt[:, 0 : gH - gC]
                                )
                            else:
                                nc.any.tensor_copy(
                                    out_m[:, gC:gH], pt[:, 0 : gH - gC]
                                )
                    if g % per == per - 1 or g == len(groups) - 1:
                        if has_bias:
                            nc.gpsimd.tensor_tensor(
                                out_m[:, col0:gH],
                                out_m[:, col0:gH],
                                bias_bc[:, col0:gH],
                                add,
                            )
                        nc.sync.dma_start(
                            out=y_d[t0 : t0 + P, col0:gH],
                            in_=out_m[:, col0:gH],
                        )
                        col0 = gH

    nc.compile()
    return nc


def _gather_bands_pe(connections, nearest_neighbors, weight):
    """Row-diagonal bands for the PE kernel, packed [128, 3*NB].

    u[i] = factor of eff[i, i-1], v[i] = eff[i, i], w[i] = eff[i, i+1]
    (per input matrix; products are computed on device).  Column d*NB + c
    holds band_d[126c + p] at partition p, zero-padded past index 4095.
    """
    NB = len(_pe_chunks())
    z1 = np.zeros(1, np.float32)

    def pack(u, v, w):
        out = np.zeros((P, 3 * NB), np.float32)
        for d, band in enumerate((u, v, w)):
            for c in range(NB):
                lo = 126 * c
                n = min(P, len(band) - lo)
                if n > 0:
                    out[:n, d * NB + c] = band[lo : lo + n]
        return out

    def bands(m, transposed):
        up = np.ascontiguousarray(np.diagonal(m, 1)).astype(np.float32, copy=False)
        mid = np.ascontiguousarray(np.diagonal(m, 0)).astype(np.float32, copy=False)
        dn = np.ascontiguousarray(np.diagonal(m, -1)).astype(np.float32, copy=False)
        if transposed:  # weight[out, in]: need w[i-1,i], w[i,i], w[i+1,i]
            u = np.concatenate([z1, up])  # weight[i-1, i] = diag(w,+1)[i-1]
            w = np.concatenate([dn, z1])  # weight[i+1, i] = diag(w,-1)[i]
        else:  # conn/nn [i, j]: need m[i, i-1], m[i, i], m[i, i+1]
            u = np.concatenate([z1, dn])  # m[i, i-1] = diag(m,-1)[i-1]
            w = np.concatenate([up, z1])  # m[i, i+1] = diag(m,+1)[i]
        return pack(u, mid, w)

    return (
        bands(connections, False),
        bands(nearest_neighbors, False),
        bands(weight, True),
    )


def _gather_bands(connections, nearest_neighbors, weight):
    """Pure indexing: extract the 3 relevant diagonals of each operand.

    Row 0 (A): entries for eff[j-1, j]  -> conn[j-1,j], nn[j-1,j], w[j,j-1]
    Row 1 (B): entries for eff[j, j]    -> conn[j,j],   nn[j,j],   w[j,j]
    Row 2 (C): entries for eff[j+1, j]  -> conn[j+1,j], nn[j+1,j], w[j,j+1]
    Out-of-range slots are zero-padded.
    """
    z1 = np.zeros(1, np.float32)

    def band3(m, transposed):
        # For conn/nn (indexed [i, j] = [row, out-col]):
        #   A[j] = m[j-1, j] = diag(m, +1) shifted;  B = diag(m, 0);
        #   C[j] = m[j+1, j] = diag(m, -1)
        # For weight (indexed [out, in] -> we need w[j, j-1], w[j,j], w[j,j+1]):
        #   A[j] = w[j, j-1] = diag(w, -1) shifted;  B = diag(w, 0);
        #   C[j] = w[j, j+1] = diag(w, +1)
        up = np.ascontiguousarray(np.diagonal(m, 1)).astype(np.float32, copy=False)
        mid = np.ascontiguousarray(np.diagonal(m, 0)).astype(np.float32, copy=False)
        dn = np.ascontiguousarray(np.diagonal(m, -1)).astype(np.float32, copy=False)
        if transposed:  # weight
            a = np.concatenate([z1, dn])
            c = np.concatenate([up, z1])
        else:  # conn / nn
            a = np.concatenate([z1, up])
            c = np.concatenate([dn, z1])
        return np.ascontiguousarray(np.stack([a, mid, c]))

    return (
        band3(connections, False),
        band3(nearest_neighbors, False),
        band3(weight, True),
    )


def kernel(x, connections, nearest_neighbors, weight, bias):
    global LAST_RESULTS
    x = np.asarray(x, dtype=np.float32)
    connections = np.asarray(connections, dtype=np.float32)
    nearest_neighbors = np.asarray(nearest_neighbors, dtype=np.float32)
    weight = np.asarray(weight, dtype=np.float32)
    bias = np.asarray(bias, dtype=np.float32)

    # Safety net: the device kernel assumes nearest_neighbors is zero
    # outside the tridiagonal band (true for this problem by construction).
    i = np.arange(FEAT)
    off_band = np.abs(i[:, None] - i[None, :]) > 1
    if np.any(nearest_neighbors[off_band] != 0.0):
        eff = connections * nearest_neighbors * weight.T
        return (x @ eff + bias).astype(np.float32)

    from concourse.bass_utils import run_bass_kernel_spmd

    has_bias = bool(np.any(bias != 0.0))
    impl = os.environ.get("KERNEL_IMPL", "pe")
    key = (impl, has_bias)
    if key not in _cached:
        builder = (
            _build_banded_pe_program if impl == "pe" else _build_banded_program
        )
        _cached[key] = builder(has_bias)
    nc = _cached[key]

    in_maps = []
    if impl == "pe":
        cb, nb, wb = _gather_bands_pe(connections, nearest_neighbors, weight)
        xT = np.ascontiguousarray(x.T)
        for c in range(N_CORES):
            m = {
                "xT": np.ascontiguousarray(
                    xT[:, c * TOK_PER_CORE : (c + 1) * TOK_PER_CORE]
                ),
                "cbT": cb,
                "nbT": nb,
                "wbT": wb,
            }
            if has_bias:
                m["bias"] = np.ascontiguousarray(bias.reshape(1, FEAT))
            in_maps.append(m)
    else:
        cb, nb, wb = _gather_bands(connections, nearest_neighbors, weight)
        for c in range(N_CORES):
            m = {
                "x": np.ascontiguousarray(
                    x[c * TOK_PER_CORE : (c + 1) * TOK_PER_CORE, :]
                ),
                "conn_band": cb,
                "nn_band": nb,
                "w_band": wb,
            }
            if has_bias:
                m["bias"] = np.ascontiguousarray(bias.reshape(1, FEAT))
            in_maps.append(m)

    trace = bool(int(os.environ.get("KERNEL_TRACE", "0")))
    res = run_bass_kernel_spmd(
        nc, in_maps, core_ids=list(range(N_CORES)), trace=trace
    )
    LAST_RESULTS = res

    out = np.empty((BATCH, FEAT), dtype=np.float32)
    for c in range(N_CORES):
        out[c * TOK_PER_CORE : (c + 1) * TOK_PER_CORE, :] = res.results[c]["y"]
    return out



# revision 6
# speedup vs baseline: 1.6377x; 1.6377x over previous
"""Trainium2 Bass kernel for NearestNeighborSparseLayer.

Reference computation:
    eff = connections * nearest_neighbors * weight.T   # [in, out]
    out = x @ eff + bias                                # [8192, 4096]

`nearest_neighbors` is a tridiagonal mask (|i-j| <= 1), so `eff` has at
most 3 nonzero diagonals and the matmul collapses to a banded (3-tap)
elementwise operation along the feature axis:

    out[t, j] = x[t, j-1]*cA[j] + x[t, j]*cB[j] + x[t, j+1]*cC[j] + bias[j]

where cA[j] = eff[j-1, j], cB[j] = eff[j, j], cC[j] = eff[j+1, j].

Strategy: data-parallel over the 8192 token rows across 8 NeuronCores
(1024 rows/core).  The host only slices/reformats data (sharding, band
gathering via np.diagonal, replication); all arithmetic — the
connections*nearest_neighbors*weight products and the banded multiply-
accumulate — runs on-device.

If `nearest_neighbors` is NOT band-limited (never the case for this
problem's input generator, which builds a tridiagonal mask), we fall
back to a plain numpy evaluation for correctness.
"""

import os

import numpy as np

BATCH = 8192
FEAT = 4096
N_CORES = 8
TOK_PER_CORE = BATCH // N_CORES  # 1024
P = 128  # partitions

LAST_RESULTS = None  # BassKernelResults from the most recent run (for test.py)

_cached = {}  # (has_bias,) -> compiled Bass program


def _build_banded_program(has_bias: bool):
    import concourse.bass as bass  # noqa: F401
    import concourse.mybir as mybir
    import concourse.tile as tile
    from concourse import bacc

    f32 = mybir.dt.float32
    mult = mybir.AluOpType.mult
    add = mybir.AluOpType.add

    nc = bacc.Bacc("TRN2", target_bir_lowering=False, debug=False)

    x_d = nc.dram_tensor("x", [TOK_PER_CORE, FEAT], f32, kind="ExternalInput").ap()
    cb_d = nc.dram_tensor("conn_band", [3, FEAT], f32, kind="ExternalInput").ap()
    nb_d = nc.dram_tensor("nn_band", [3, FEAT], f32, kind="ExternalInput").ap()
    wb_d = nc.dram_tensor("w_band", [3, FEAT], f32, kind="ExternalInput").ap()
    if has_bias:
        bias_d = nc.dram_tensor("bias", [1, FEAT], f32, kind="ExternalInput").ap()
    y_d = nc.dram_tensor("y", [TOK_PER_CORE, FEAT], f32, kind="ExternalOutput").ap()

    n_tiles = TOK_PER_CORE // P  # 8

    # bands live as [96, 128] tiles (3*4096 elements spread over 96
    # partitions) so they cost 512B/partition instead of 16KB/partition
    bp, bf = 96, 128

    with tile.TileContext(nc) as tc:
        with (
            tc.tile_pool(name="const", bufs=1) as const,
            tc.tile_pool(name="xp", bufs=2) as xp,
            tc.tile_pool(name="tp", bufs=2) as tp,
            tc.tile_pool(name="dram", bufs=1, space="DRAM") as dram,
        ):
            # --- one-time: compute banded coefficients on device ---
            cb_sb = const.tile([bp, bf], f32, tag="cb")
            nb_sb = const.tile([bp, bf], f32, tag="nb")
            wb_sb = const.tile([bp, bf], f32, tag="wb")
            r96 = lambda ap: ap.rearrange("a (b c) -> (a b) c", c=bf)
            nc.sync.dma_start(out=cb_sb[:], in_=r96(cb_d))
            nc.sync.dma_start(out=nb_sb[:], in_=r96(nb_d))
            nc.sync.dma_start(out=wb_sb[:], in_=r96(wb_d))
            coef = const.tile([bp, bf], f32, tag="coef")
            nc.vector.tensor_tensor(coef[:], cb_sb[:], nb_sb[:], mult)
            nc.vector.tensor_tensor(coef[:], coef[:], wb_sb[:], mult)

            # round-trip through DRAM so we can broadcast each row across
            # all 128 partitions with a step-0 DMA read
            coef_dram = dram.tile([3, FEAT], f32, tag="coefd")
            nc.sync.dma_start(out=r96(coef_dram[:]), in_=coef[:])

            A = const.tile([P, FEAT], f32, tag="A")
            B = const.tile([P, FEAT], f32, tag="B")
            C = const.tile([P, FEAT], f32, tag="C")
            nc.sync.dma_start(out=A[:], in_=coef_dram[0:1, :].broadcast_to([P, FEAT]))
            nc.sync.dma_start(out=B[:], in_=coef_dram[1:2, :].broadcast_to([P, FEAT]))
            nc.sync.dma_start(out=C[:], in_=coef_dram[2:3, :].broadcast_to([P, FEAT]))
            if has_bias:
                BI = const.tile([P, FEAT], f32, tag="BI")
                nc.sync.dma_start(
                    out=BI[:], in_=bias_d[0:1, :].broadcast_to([P, FEAT])
                )

            # --- main loop: banded 3-tap multiply-accumulate ---
            for i in range(n_tiles):
                r0 = i * P
                xt = xp.tile([P, FEAT + 2], f32, tag="x")
                nc.vector.memset(xt[:, 0:1], 0.0)
                nc.vector.memset(xt[:, FEAT + 1 : FEAT + 2], 0.0)
                nc.sync.dma_start(out=xt[:, 1 : FEAT + 1], in_=x_d[r0 : r0 + P, :])

                t_a = tp.tile([P, FEAT], f32, tag="ta")
                t_b = tp.tile([P, FEAT], f32, tag="tb")
                t_c = tp.tile([P, FEAT], f32, tag="tc")

                # x[t, j-1] * cA[j]
                nc.vector.tensor_tensor(t_a[:], xt[:, 0:FEAT], A[:], mult)
                # x[t, j+1] * cC[j]
                nc.vector.tensor_tensor(t_c[:], xt[:, 2 : FEAT + 2], C[:], mult)
                # x[t, j] * cB[j]   (gpsimd runs in parallel with DVE)
                nc.gpsimd.tensor_tensor(t_b[:], xt[:, 1 : FEAT + 1], B[:], mult)
                # t_a += t_c  (in-place: identical in/out APs are safe for
                # elementwise streaming ops)
                nc.vector.tensor_tensor(t_a[:], t_a[:], t_c[:], add)
                if has_bias:
                    nc.gpsimd.tensor_tensor(t_b[:], t_b[:], BI[:], add)
                nc.gpsimd.tensor_tensor(t_b[:], t_a[:], t_b[:], add)

                nc.sync.dma_start(out=y_d[r0 : r0 + P, :], in_=t_b[:])

    nc.compile()
    return nc


def _pe_chunks():
    """Non-overlapping column chunks for the PE-banded kernel.

    Chunk c produces output columns [C_c, C_c + N_c) from input rows
    [R_c, R_c + K_c), where the 3-diagonal band makes each column depend on
    rows col-1..col+1.  With R_c = 126*c the row windows fit in 128
    partitions and every output column is produced by exactly ONE matmul
    (no PSUM accumulation).  delta = C_c - R_c selects which diagonals of
    the rhs block are populated.

    Returns list of (c, R, K, C, N, delta).
    """
    chunks = []
    c = 0
    col = 0
    while col < FEAT:
        R = 126 * c
        K = min(P, FEAT - R)
        delta = col - R  # 0 for chunk 0, 1 afterwards
        max_col = FEAT - 1 if R + K >= FEAT else R + K - 2
        N = max_col - col + 1
        chunks.append((c, R, K, col, N, delta))
        col += N
        c += 1
    return chunks


def _build_banded_pe_program(has_bias: bool):
    """v2: banded matmul on the tensor engine, non-overlapping chunks.

    For each chunk (R, K, C, N, delta):
        out[tokens, C:C+N] = xT[R:R+K, tokens].T @ E_c[0:K, 0:N]
    where E_c is the dense banded block of eff rows R..R+K-1 x cols
    C..C+N-1, built on device from the gathered diagonals.  Every output
    column is produced by exactly one matmul (start=stop=True), so no
    PSUM accumulation semantics are needed.
    """
    import concourse.bass as bass  # noqa: F401
    import concourse.mybir as mybir
    import concourse.tile as tile
    from concourse import bacc

    f32 = mybir.dt.float32
    mult = mybir.AluOpType.mult
    add = mybir.AluOpType.add

    nc = bacc.Bacc("TRN2", target_bir_lowering=False, debug=False)

    chunks = _pe_chunks()
    n_chunks = len(chunks)  # 33
    n_m = TOK_PER_CORE // P  # 8
    NB = n_chunks  # band columns per diagonal

    xT_d = nc.dram_tensor("xT", [FEAT, TOK_PER_CORE], f32, kind="ExternalInput").ap()
    # bands packed [128, 3*NB]: col d*NB + c holds band_d[126c + p] at
    # partition p (d: 0=u sub, 1=v main, 2=w super diag of eff's rows)
    cb_d = nc.dram_tensor("cbT", [P, 3 * NB], f32, kind="ExternalInput").ap()
    nb_d = nc.dram_tensor("nbT", [P, 3 * NB], f32, kind="ExternalInput").ap()
    wb_d = nc.dram_tensor("wbT", [P, 3 * NB], f32, kind="ExternalInput").ap()
    if has_bias:
        bias_d = nc.dram_tensor("bias", [1, FEAT], f32, kind="ExternalInput").ap()
    y_d = nc.dram_tensor("y", [TOK_PER_CORE, FEAT], f32, kind="ExternalOutput").ap()

    with tile.TileContext(nc) as tc:
        with (
            tc.tile_pool(name="const", bufs=1) as const,
            tc.tile_pool(name="xp", bufs=1) as xp,
            tc.tile_pool(name="op", bufs=int(os.environ.get("KERNEL_OPBUFS", "2"))) as op,
            tc.tile_pool(name="pp", bufs=8, space="PSUM") as pp,
        ):
            # IDW[p, q] = 1 iff p == q-1; slicing IDW[:, d+1 : d+1+N] gives
            # the shifted identity J_d[p, q] = [p == q+d] for d in -1..2
            idw = const.tile([P, P + 2], f32, tag="idw")
            nc.gpsimd.memset(idw[:], 0.0)
            nc.gpsimd.affine_select(
                out=idw[:],
                in_=idw[:],
                compare_op=mybir.AluOpType.not_equal,
                fill=1.0,
                base=1,
                # fill where (p - q + 1) == 0, i.e. at q = p+1
                pattern=[[-1, P + 2]],
                channel_multiplier=1,
            )

            cb_sb = const.tile([P, 3 * NB], f32, tag="cb")
            nb_sb = const.tile([P, 3 * NB], f32, tag="nb")
            wb_sb = const.tile([P, 3 * NB], f32, tag="wb")
            nc.sync.dma_start(out=cb_sb[:], in_=cb_d[:])
            nc.sync.dma_start(out=nb_sb[:], in_=nb_d[:])
            nc.sync.dma_start(out=wb_sb[:], in_=wb_d[:])
            uvw = const.tile([P, 3 * NB], f32, tag="uvw")
            nc.vector.tensor_tensor(uvw[:], cb_sb[:], nb_sb[:], mult)
            nc.vector.tensor_tensor(uvw[:], uvw[:], wb_sb[:], mult)

            if has_bias:
                bias_bc = const.tile([P, FEAT], f32, tag="biasbc")
                nc.sync.dma_start(
                    out=bias_bc[:], in_=bias_d[0:1, :].broadcast_to([P, FEAT])
                )

            def jd(d, n):  # shifted identity J_d [128, n]
                return idw[:, d + 1 : d + 1 + n]

            def sv(d, c):  # per-partition band scalar for diag d, chunk c
                return uvw[:, d * NB + c : d * NB + c + 1]

            # E_c[p, q] = eff[R+p, C+q]: diag d=p-q==delta-1 -> w[R+p],
            # ==delta -> v[R+p], ==delta+1 -> u[R+p]
            eblocks = []
            for c, R, K, C, N, delta in chunks:
                E = const.tile([P, P + 1], f32, tag=f"E{c}", name=f"E{c}")
                nc.vector.tensor_scalar(
                    E[:, 0:N], jd(delta - 1, N), sv(2, c), None, mult
                )
                nc.vector.scalar_tensor_tensor(
                    E[:, 0:N], jd(delta, N), sv(1, c), E[:, 0:N], mult, add
                )
                nc.vector.scalar_tensor_tensor(
                    E[:, 0:N], jd(delta + 1, N), sv(0, c), E[:, 0:N], mult, add
                )
                eblocks.append(E)

            # whole xT shard in SBUF once, as 33 overlapping row-slabs
            # [K, 1024] (~132KB/partition); reused by all 8 m-blocks
            X = xp.tile([P, n_chunks, TOK_PER_CORE], f32, tag="X")
            for c, R, K, C, N, delta in chunks:
                nc.sync.dma_start(out=X[0:K, c, :], in_=xT_d[R : R + K, :])

            ablate = os.environ.get("KERNEL_ABLATE", "")
            # chunks grouped 4-per-PSUM-bank: the first matmul in a group
            # arms the 2KB bank (start=True); later matmuls overwrite their
            # own still-pending columns; one copy evicts the whole group.
            GRP = int(os.environ.get("KERNEL_GRP", "1"))
            groups = [chunks[i : i + GRP] for i in range(0, n_chunks, GRP)]
            # out DMA piece boundaries, in units of groups
            per = int(os.environ.get("KERNEL_PIECE_GROUPS", "0")) or max(1, len(chunks) // (4 * GRP))
            cmode = os.environ.get("KERNEL_COPY", "a")
            for m in range(n_m):
                t0 = m * P
                out_m = op.tile([P, FEAT], f32, tag="out")
                if ablate:
                    nc.vector.memset(out_m[:, 0:1], 0.0)
                col0 = 0
                for g, grp in enumerate(groups):
                    gC = grp[0][3]  # first col of group
                    gH = grp[-1][3] + grp[-1][4]  # end col
                    if "nomm" not in ablate:
                        pt = pp.tile([P, 512], f32, tag="ps", name=f"ps_{m}_{g}")
                        for j, (c, R, K, C, N, delta) in enumerate(grp):
                            nc.tensor.matmul(
                                pt[0:P, C - gC : C - gC + N],
                                X[0:K, c, t0 : t0 + P],
                                eblocks[c][0:K, 0:N],
                                start=(j == 0),
                                stop=(j == len(grp) - 1),
                            )
                        if "nocopy" not in ablate:
                            eng = [ch for ch in cmode][g % len(cmode)]
                            if eng == "v":
                                nc.vector.tensor_copy(
                                    out_m[:, gC:gH], pt[:, 0 : gH - gC]
                                )
                            elif eng == "s":
                                nc.scalar.copy(
                                    out_m[:, gC:gH], pt[:, 0 : gH - gC]
                                )
                            else:
                                nc.any.tensor_copy(
                                    out_m[:, gC:gH], pt[:, 0 : gH - gC]
                                )
                    if g % per == per - 1 or g == len(groups) - 1:
                        if has_bias:
                            nc.gpsimd.tensor_tensor(
                                out_m[:, col0:gH],
                                out_m[:, col0:gH],
                                bias_bc[:, col0:gH],
                                add,
                            )
                        nc.sync.dma_start(
                            out=y_d[t0 : t0 + P, col0:gH],
                            in_=out_m[:, col0:gH],
                        )
                        col0 = gH

    nc.compile()
    return nc


def _build_banded_pe16_program(has_bias: bool):
    """v3: fp16 I/O banded matmul on the tensor engine.

    Same chunk geometry as the fp32 PE kernel, but x is streamed in as
    fp16 and y written back as fp16 (host casts to/from fp32; the 2e-2
    rel-err budget dwarfs fp16's ~5e-4).  This halves HBM traffic — the
    sole bottleneck — and makes every matmul 1 cycle/row instead of 4.

    Structure per core:
      - bands [128, 3*NB] fp32 -> uvw products -> cast fp16
      - E blocks (banded rhs) built fp16 via shifted-identity selects
      - X: whole fp16 xT shard resident in SBUF ([128, 33, 1024], 66KB/par)
      - 8 m-blocks x 9 chunk-groups: 4 matmuls/group accumulate-free into
        one PSUM bank, then one cast-copy (alternating DVE/ACT) into the
        fp16 out tile; one DMA store per m-block from the Pool queue so
        stores never stall the SP load queue.
    """
    import concourse.bass as bass  # noqa: F401
    import concourse.mybir as mybir
    import concourse.tile as tile
    from concourse import bacc

    f32 = mybir.dt.float32
    f16 = mybir.dt.float16
    mult = mybir.AluOpType.mult
    add = mybir.AluOpType.add

    nc = bacc.Bacc("TRN2", target_bir_lowering=False, debug=False)

    chunks = _pe_chunks()
    NB = len(chunks)  # 33
    n_m = TOK_PER_CORE // P  # 8

    xT_d = nc.dram_tensor("xT", [FEAT, TOK_PER_CORE], f16, kind="ExternalInput").ap()
    cb_d = nc.dram_tensor("cbT", [P, 3 * NB], f32, kind="ExternalInput").ap()
    nb_d = nc.dram_tensor("nbT", [P, 3 * NB], f32, kind="ExternalInput").ap()
    wb_d = nc.dram_tensor("wbT", [P, 3 * NB], f32, kind="ExternalInput").ap()
    if has_bias:
        bias_d = nc.dram_tensor("bias", [1, FEAT], f32, kind="ExternalInput").ap()
    y_d = nc.dram_tensor("y", [TOK_PER_CORE, FEAT], f16, kind="ExternalOutput").ap()

    GRP = int(os.environ.get("KERNEL_GRP", "4"))
    groups = [chunks[i : i + GRP] for i in range(0, NB, GRP)]

    with tile.TileContext(nc) as tc:
        with (
            tc.tile_pool(name="const", bufs=1) as const,
            tc.tile_pool(name="xp", bufs=1) as xp,
            tc.tile_pool(name="op", bufs=2) as op,
            tc.tile_pool(name="pp", bufs=8, space="PSUM") as pp,
        ):
            # IDW[p, q] = 1 iff p == q-1 (fp16); IDW[:, d+1 : d+1+N] is the
            # shifted identity J_d[p, q] = [p == q+d] for d in -1..2
            idw = const.tile([P, P + 2], f16, tag="idw")
            nc.gpsimd.memset(idw[:], 0.0)
            nc.gpsimd.affine_select(
                out=idw[:],
                in_=idw[:],
                compare_op=mybir.AluOpType.not_equal,
                fill=1.0,
                base=1,
                pattern=[[-1, P + 2]],
                channel_multiplier=1,
            )

            # band loads go out on the ACT queue so the SP queue starts the
            # big X streams immediately
            cb_sb = const.tile([P, 3 * NB], f32, tag="cb")
            nb_sb = const.tile([P, 3 * NB], f32, tag="nb")
            wb_sb = const.tile([P, 3 * NB], f32, tag="wb")
            nc.scalar.dma_start(out=cb_sb[:], in_=cb_d[:])
            nc.scalar.dma_start(out=nb_sb[:], in_=nb_d[:])
            nc.scalar.dma_start(out=wb_sb[:], in_=wb_d[:])
            uvw = const.tile([P, 3 * NB], f32, tag="uvw")
            nc.vector.tensor_tensor(uvw[:], cb_sb[:], nb_sb[:], mult)
            nc.vector.tensor_tensor(uvw[:], uvw[:], wb_sb[:], mult)

            if has_bias:
                bias_bc = const.tile([P, FEAT], f32, tag="biasbc")
                nc.scalar.dma_start(
                    out=bias_bc[:], in_=bias_d[0:1, :].broadcast_to([P, FEAT])
                )

            def jd(d, n):  # shifted identity J_d [128, n]
                return idw[:, d + 1 : d + 1 + n]

            def sv(d, c):  # per-partition band scalar for diag d, chunk c
                return uvw[:, d * NB + c : d * NB + c + 1]

            # E_c[p, q] = eff[R+p, C+q]: diag d=p-q==delta-1 -> w[R+p],
            # ==delta -> v[R+p], ==delta+1 -> u[R+p]
            eblocks = []
            for c, R, K, C, N, delta in chunks:
                E = const.tile([P, P + 1], f16, tag=f"E{c}", name=f"E{c}")
                nc.vector.tensor_scalar(
                    E[:, 0:N], jd(delta - 1, N), sv(2, c), None, mult
                )
                nc.vector.scalar_tensor_tensor(
                    E[:, 0:N], jd(delta, N), sv(1, c), E[:, 0:N], mult, add
                )
                nc.vector.scalar_tensor_tensor(
                    E[:, 0:N], jd(delta + 1, N), sv(0, c), E[:, 0:N], mult, add
                )
                eblocks.append(E)

            # whole fp16 xT shard in SBUF as 33 overlapping row-slabs
            X = xp.tile([P, NB, TOK_PER_CORE], f16, tag="X")
            for c, R, K, C, N, delta in chunks:
                nc.sync.dma_start(out=X[0:K, c, :], in_=xT_d[R : R + K, :])

            ncopy = 0
            for m in range(n_m):
                t0 = m * P
                out_m = op.tile([P, FEAT], f16, tag="out")
                for g, grp in enumerate(groups):
                    gC = grp[0][3]  # first col of group
                    gH = grp[-1][3] + grp[-1][4]  # end col
                    pt = pp.tile([P, 512], f32, tag="ps", name=f"ps_{m}_{g}")
                    for j, (c, R, K, C, N, delta) in enumerate(grp):
                        nc.tensor.matmul(
                            pt[0:P, C - gC : C - gC + N],
                            X[0:K, c, t0 : t0 + P],
                            eblocks[c][0:K, 0:N],
                            start=(j == 0),
                            stop=(j == len(grp) - 1),
                        )
                    # PSUM->SBUF cast copy, alternating DVE/ACT
                    if ncopy % 2 == 0:
                        nc.vector.tensor_copy(out_m[:, gC:gH], pt[:, 0 : gH - gC])
                    else:
                        nc.scalar.copy(out_m[:, gC:gH], pt[:, 0 : gH - gC])
                    ncopy += 1
                if has_bias:
                    nc.gpsimd.tensor_tensor(
                        out_m[:], out_m[:], bias_bc[:], add
                    )
                # store from the Pool queue: never blocks the SP load queue
                nc.gpsimd.dma_start(out=y_d[t0 : t0 + P, :], in_=out_m[:])

    nc.compile()
    return nc


def _gather_bands_pe(connections, nearest_neighbors, weight):
    """Row-diagonal bands for the PE kernel, packed [128, 3*NB].

    u[i] = factor of eff[i, i-1], v[i] = eff[i, i], w[i] = eff[i, i+1]
    (per input matrix; products are computed on device).  Column d*NB + c
    holds band_d[126c + p] at partition p, zero-padded past index 4095.
    """
    NB = len(_pe_chunks())
    z1 = np.zeros(1, np.float32)

    def pack(u, v, w):
        out = np.zeros((P, 3 * NB), np.float32)
        for d, band in enumerate((u, v, w)):
            for c in range(NB):
                lo = 126 * c
                n = min(P, len(band) - lo)
                if n > 0:
                    out[:n, d * NB + c] = band[lo : lo + n]
        return out

    def bands(m, transposed):
        up = np.ascontiguousarray(np.diagonal(m, 1)).astype(np.float32, copy=False)
        mid = np.ascontiguousarray(np.diagonal(m, 0)).astype(np.float32, copy=False)
        dn = np.ascontiguousarray(np.diagonal(m, -1)).astype(np.float32, copy=False)
        if transposed:  # weight[out, in]: need w[i-1,i], w[i,i], w[i+1,i]
            u = np.concatenate([z1, up])  # weight[i-1, i] = diag(w,+1)[i-1]
            w = np.concatenate([dn, z1])  # weight[i+1, i] = diag(w,-1)[i]
        else:  # conn/nn [i, j]: need m[i, i-1], m[i, i], m[i, i+1]
            u = np.concatenate([z1, dn])  # m[i, i-1] = diag(m,-1)[i-1]
            w = np.concatenate([up, z1])  # m[i, i+1] = diag(m,+1)[i]
        return pack(u, mid, w)

    return (
        bands(connections, False),
        bands(nearest_neighbors, False),
        bands(weight, True),
    )


def _gather_bands(connections, nearest_neighbors, weight):
    """Pure indexing: extract the 3 relevant diagonals of each operand.

    Row 0 (A): entries for eff[j-1, j]  -> conn[j-1,j], nn[j-1,j], w[j,j-1]
    Row 1 (B): entries for eff[j, j]    -> conn[j,j],   nn[j,j],   w[j,j]
    Row 2 (C): entries for eff[j+1, j]  -> conn[j+1,j], nn[j+1,j], w[j,j+1]
    Out-of-range slots are zero-padded.
    """
    z1 = np.zeros(1, np.float32)

    def band3(m, transposed):
        # For conn/nn (indexed [i, j] = [row, out-col]):
        #   A[j] = m[j-1, j] = diag(m, +1) shifted;  B = diag(m, 0);
        #   C[j] = m[j+1, j] = diag(m, -1)
        # For weight (indexed [out, in] -> we need w[j, j-1], w[j,j], w[j,j+1]):
        #   A[j] = w[j, j-1] = diag(w, -1) shifted;  B = diag(w, 0);
        #   C[j] = w[j, j+1] = diag(w, +1)
        up = np.ascontiguousarray(np.diagonal(m, 1)).astype(np.float32, copy=False)
        mid = np.ascontiguousarray(np.diagonal(m, 0)).astype(np.float32, copy=False)
        dn = np.ascontiguousarray(np.diagonal(m, -1)).astype(np.float32, copy=False)
        if transposed:  # weight
            a = np.concatenate([z1, dn])
            c = np.concatenate([up, z1])
        else:  # conn / nn
            a = np.concatenate([z1, up])
            c = np.concatenate([dn, z1])
        return np.ascontiguousarray(np.stack([a, mid, c]))

    return (
        band3(connections, False),
        band3(nearest_neighbors, False),
        band3(weight, True),
    )


def kernel(x, connections, nearest_neighbors, weight, bias):
    global LAST_RESULTS
    x = np.asarray(x, dtype=np.float32)
    connections = np.asarray(connections, dtype=np.float32)
    nearest_neighbors = np.asarray(nearest_neighbors, dtype=np.float32)
    weight = np.asarray(weight, dtype=np.float32)
    bias = np.asarray(bias, dtype=np.float32)

    # Safety net: the device kernel assumes nearest_neighbors is zero
    # outside the tridiagonal band (true for this problem by construction).
    i = np.arange(FEAT)
    off_band = np.abs(i[:, None] - i[None, :]) > 1
    if np.any(nearest_neighbors[off_band] != 0.0):
        eff = connections * nearest_neighbors * weight.T
        return (x @ eff + bias).astype(np.float32)

    from concourse.bass_utils import run_bass_kernel_spmd

    has_bias = bool(np.any(bias != 0.0))
    impl = os.environ.get("KERNEL_IMPL", "pe16")
    key = (impl, has_bias)
    if key not in _cached:
        builder = {
            "pe16": _build_banded_pe16_program,
            "pe": _build_banded_pe_program,
            "banded": _build_banded_program,
        }[impl]
        _cached[key] = builder(has_bias)
    nc = _cached[key]

    in_maps = []
    if impl == "pe16":
        cb, nb, wb = _gather_bands_pe(connections, nearest_neighbors, weight)
        xT = x.astype(np.float16).T  # [FEAT, BATCH] view of the fp16 cast
        for c in range(N_CORES):
            m = {
                "xT": np.ascontiguousarray(
                    xT[:, c * TOK_PER_CORE : (c + 1) * TOK_PER_CORE]
                ),
                "cbT": cb,
                "nbT": nb,
                "wbT": wb,
            }
            if has_bias:
                m["bias"] = np.ascontiguousarray(bias.reshape(1, FEAT))
            in_maps.append(m)
    elif impl == "pe":
        cb, nb, wb = _gather_bands_pe(connections, nearest_neighbors, weight)
        xT = np.ascontiguousarray(x.T)
        for c in range(N_CORES):
            m = {
                "xT": np.ascontiguousarray(
                    xT[:, c * TOK_PER_CORE : (c + 1) * TOK_PER_CORE]
                ),
                "cbT": cb,
                "nbT": nb,
                "wbT": wb,
            }
            if has_bias:
                m["bias"] = np.ascontiguousarray(bias.reshape(1, FEAT))
            in_maps.append(m)
    else:
        cb, nb, wb = _gather_bands(connections, nearest_neighbors, weight)
        for c in range(N_CORES):
            m = {
                "x": np.ascontiguousarray(
                    x[c * TOK_PER_CORE : (c + 1) * TOK_PER_CORE, :]
                ),
                "conn_band": cb,
                "nn_band": nb,
                "w_band": wb,
            }
            if has_bias:
                m["bias"] = np.ascontiguousarray(bias.reshape(1, FEAT))
            in_maps.append(m)

    trace = bool(int(os.environ.get("KERNEL_TRACE", "0")))
    res = run_bass_kernel_spmd(
        nc, in_maps, core_ids=list(range(N_CORES)), trace=trace
    )
    LAST_RESULTS = res

    out = np.empty((BATCH, FEAT), dtype=np.float32)
    for c in range(N_CORES):
        yc = res.results[c]["y"]
        out[c * TOK_PER_CORE : (c + 1) * TOK_PER_CORE, :] = np.asarray(
            yc, dtype=np.float32
        )
    return out



# revision 17
# speedup vs baseline: 1.8703x; 1.1420x over previous
"""Trainium2 Bass kernel for NearestNeighborSparseLayer.

Reference computation:
    eff = connections * nearest_neighbors * weight.T   # [in, out]
    out = x @ eff + bias                                # [8192, 4096]

`nearest_neighbors` is a tridiagonal mask (|i-j| <= 1), so `eff` has at
most 3 nonzero diagonals and the matmul collapses to a banded (3-tap)
elementwise operation along the feature axis:

    out[t, j] = x[t, j-1]*cA[j] + x[t, j]*cB[j] + x[t, j+1]*cC[j] + bias[j]

where cA[j] = eff[j-1, j], cB[j] = eff[j, j], cC[j] = eff[j+1, j].

Strategy: data-parallel over the 8192 token rows across 8 NeuronCores
(1024 rows/core).  The host only slices/reformats data (sharding, band
gathering via np.diagonal, replication); all arithmetic — the
connections*nearest_neighbors*weight products and the banded multiply-
accumulate — runs on-device.

If `nearest_neighbors` is NOT band-limited (never the case for this
problem's input generator, which builds a tridiagonal mask), we fall
back to a plain numpy evaluation for correctness.
"""

import os

import numpy as np

BATCH = 8192
FEAT = 4096
N_CORES = 8
TOK_PER_CORE = BATCH // N_CORES  # 1024
P = 128  # partitions

LAST_RESULTS = None  # BassKernelResults from the most recent run (for test.py)

_cached = {}  # (has_bias,) -> compiled Bass program


def _build_banded_program(has_bias: bool):
    import concourse.bass as bass  # noqa: F401
    import concourse.mybir as mybir
    import concourse.tile as tile
    from concourse import bacc

    f32 = mybir.dt.float32
    mult = mybir.AluOpType.mult
    add = mybir.AluOpType.add

    nc = bacc.Bacc("TRN2", target_bir_lowering=False, debug=False)

    x_d = nc.dram_tensor("x", [TOK_PER_CORE, FEAT], f32, kind="ExternalInput").ap()
    cb_d = nc.dram_tensor("conn_band", [3, FEAT], f32, kind="ExternalInput").ap()
    nb_d = nc.dram_tensor("nn_band", [3, FEAT], f32, kind="ExternalInput").ap()
    wb_d = nc.dram_tensor("w_band", [3, FEAT], f32, kind="ExternalInput").ap()
    if has_bias:
        bias_d = nc.dram_tensor("bias", [1, FEAT], f32, kind="ExternalInput").ap()
    y_d = nc.dram_tensor("y", [TOK_PER_CORE, FEAT], f32, kind="ExternalOutput").ap()

    n_tiles = TOK_PER_CORE // P  # 8

    # bands live as [96, 128] tiles (3*4096 elements spread over 96
    # partitions) so they cost 512B/partition instead of 16KB/partition
    bp, bf = 96, 128

    with tile.TileContext(nc) as tc:
        with (
            tc.tile_pool(name="const", bufs=1) as const,
            tc.tile_pool(name="xp", bufs=2) as xp,
            tc.tile_pool(name="tp", bufs=2) as tp,
            tc.tile_pool(name="dram", bufs=1, space="DRAM") as dram,
        ):
            # --- one-time: compute banded coefficients on device ---
            cb_sb = const.tile([bp, bf], f32, tag="cb")
            nb_sb = const.tile([bp, bf], f32, tag="nb")
            wb_sb = const.tile([bp, bf], f32, tag="wb")
            r96 = lambda ap: ap.rearrange("a (b c) -> (a b) c", c=bf)
            nc.sync.dma_start(out=cb_sb[:], in_=r96(cb_d))
            nc.sync.dma_start(out=nb_sb[:], in_=r96(nb_d))
            nc.sync.dma_start(out=wb_sb[:], in_=r96(wb_d))
            coef = const.tile([bp, bf], f32, tag="coef")
            nc.vector.tensor_tensor(coef[:], cb_sb[:], nb_sb[:], mult)
            nc.vector.tensor_tensor(coef[:], coef[:], wb_sb[:], mult)

            # round-trip through DRAM so we can broadcast each row across
            # all 128 partitions with a step-0 DMA read
            coef_dram = dram.tile([3, FEAT], f32, tag="coefd")
            nc.sync.dma_start(out=r96(coef_dram[:]), in_=coef[:])

            A = const.tile([P, FEAT], f32, tag="A")
            B = const.tile([P, FEAT], f32, tag="B")
            C = const.tile([P, FEAT], f32, tag="C")
            nc.sync.dma_start(out=A[:], in_=coef_dram[0:1, :].broadcast_to([P, FEAT]))
            nc.sync.dma_start(out=B[:], in_=coef_dram[1:2, :].broadcast_to([P, FEAT]))
            nc.sync.dma_start(out=C[:], in_=coef_dram[2:3, :].broadcast_to([P, FEAT]))
            if has_bias:
                BI = const.tile([P, FEAT], f32, tag="BI")
                nc.sync.dma_start(
                    out=BI[:], in_=bias_d[0:1, :].broadcast_to([P, FEAT])
                )

            # --- main loop: banded 3-tap multiply-accumulate ---
            for i in range(n_tiles):
                r0 = i * P
                xt = xp.tile([P, FEAT + 2], f32, tag="x")
                nc.vector.memset(xt[:, 0:1], 0.0)
                nc.vector.memset(xt[:, FEAT + 1 : FEAT + 2], 0.0)
                nc.sync.dma_start(out=xt[:, 1 : FEAT + 1], in_=x_d[r0 : r0 + P, :])

                t_a = tp.tile([P, FEAT], f32, tag="ta")
                t_b = tp.tile([P, FEAT], f32, tag="tb")
                t_c = tp.tile([P, FEAT], f32, tag="tc")

                # x[t, j-1] * cA[j]
                nc.vector.tensor_tensor(t_a[:], xt[:, 0:FEAT], A[:], mult)
                # x[t, j+1] * cC[j]
                nc.vector.tensor_tensor(t_c[:], xt[:, 2 : FEAT + 2], C[:], mult)
                # x[t, j] * cB[j]   (gpsimd runs in parallel with DVE)
                nc.gpsimd.tensor_tensor(t_b[:], xt[:, 1 : FEAT + 1], B[:], mult)
                # t_a += t_c  (in-place: identical in/out APs are safe for
                # elementwise streaming ops)
                nc.vector.tensor_tensor(t_a[:], t_a[:], t_c[:], add)
                if has_bias:
                    nc.gpsimd.tensor_tensor(t_b[:], t_b[:], BI[:], add)
                nc.gpsimd.tensor_tensor(t_b[:], t_a[:], t_b[:], add)

                nc.sync.dma_start(out=y_d[r0 : r0 + P, :], in_=t_b[:])

    nc.compile()
    return nc


def _pe_chunks():
    """Non-overlapping column chunks for the PE-banded kernel.

    Chunk c produces output columns [C_c, C_c + N_c) from input rows
    [R_c, R_c + K_c), where the 3-diagonal band makes each column depend on
    rows col-1..col+1.  With R_c = 126*c the row windows fit in 128
    partitions and every output column is produced by exactly ONE matmul
    (no PSUM accumulation).  delta = C_c - R_c selects which diagonals of
    the rhs block are populated.

    Returns list of (c, R, K, C, N, delta).
    """
    chunks = []
    c = 0
    col = 0
    while col < FEAT:
        R = 126 * c
        K = min(P, FEAT - R)
        delta = col - R  # 0 for chunk 0, 1 afterwards
        max_col = FEAT - 1 if R + K >= FEAT else R + K - 2
        N = max_col - col + 1
        chunks.append((c, R, K, col, N, delta))
        col += N
        c += 1
    return chunks


def _build_banded_pe_program(has_bias: bool):
    """v2: banded matmul on the tensor engine, non-overlapping chunks.

    For each chunk (R, K, C, N, delta):
        out[tokens, C:C+N] = xT[R:R+K, tokens].T @ E_c[0:K, 0:N]
    where E_c is the dense banded block of eff rows R..R+K-1 x cols
    C..C+N-1, built on device from the gathered diagonals.  Every output
    column is produced by exactly one matmul (start=stop=True), so no
    PSUM accumulation semantics are needed.
    """
    import concourse.bass as bass  # noqa: F401
    import concourse.mybir as mybir
    import concourse.tile as tile
    from concourse import bacc

    f32 = mybir.dt.float32
    mult = mybir.AluOpType.mult
    add = mybir.AluOpType.add

    nc = bacc.Bacc("TRN2", target_bir_lowering=False, debug=False)

    chunks = _pe_chunks()
    n_chunks = len(chunks)  # 33
    n_m = TOK_PER_CORE // P  # 8
    NB = n_chunks  # band columns per diagonal

    xT_d = nc.dram_tensor("xT", [FEAT, TOK_PER_CORE], f32, kind="ExternalInput").ap()
    # bands packed [128, 3*NB]: col d*NB + c holds band_d[126c + p] at
    # partition p (d: 0=u sub, 1=v main, 2=w super diag of eff's rows)
    cb_d = nc.dram_tensor("cbT", [P, 3 * NB], f32, kind="ExternalInput").ap()
    nb_d = nc.dram_tensor("nbT", [P, 3 * NB], f32, kind="ExternalInput").ap()
    wb_d = nc.dram_tensor("wbT", [P, 3 * NB], f32, kind="ExternalInput").ap()
    if has_bias:
        bias_d = nc.dram_tensor("bias", [1, FEAT], f32, kind="ExternalInput").ap()
    y_d = nc.dram_tensor("y", [TOK_PER_CORE, FEAT], f32, kind="ExternalOutput").ap()

    with tile.TileContext(nc) as tc:
        with (
            tc.tile_pool(name="const", bufs=1) as const,
            tc.tile_pool(name="xp", bufs=1) as xp,
            tc.tile_pool(name="op", bufs=int(os.environ.get("KERNEL_OPBUFS", "2"))) as op,
            tc.tile_pool(name="pp", bufs=8, space="PSUM") as pp,
        ):
            # IDW[p, q] = 1 iff p == q-1; slicing IDW[:, d+1 : d+1+N] gives
            # the shifted identity J_d[p, q] = [p == q+d] for d in -1..2
            idw = const.tile([P, P + 2], f32, tag="idw")
            nc.gpsimd.memset(idw[:], 0.0)
            nc.gpsimd.affine_select(
                out=idw[:],
                in_=idw[:],
                compare_op=mybir.AluOpType.not_equal,
                fill=1.0,
                base=1,
                # fill where (p - q + 1) == 0, i.e. at q = p+1
                pattern=[[-1, P + 2]],
                channel_multiplier=1,
            )

            cb_sb = const.tile([P, 3 * NB], f32, tag="cb")
            nb_sb = const.tile([P, 3 * NB], f32, tag="nb")
            wb_sb = const.tile([P, 3 * NB], f32, tag="wb")
            nc.sync.dma_start(out=cb_sb[:], in_=cb_d[:])
            nc.sync.dma_start(out=nb_sb[:], in_=nb_d[:])
            nc.sync.dma_start(out=wb_sb[:], in_=wb_d[:])
            uvw = const.tile([P, 3 * NB], f32, tag="uvw")
            nc.vector.tensor_tensor(uvw[:], cb_sb[:], nb_sb[:], mult)
            nc.vector.tensor_tensor(uvw[:], uvw[:], wb_sb[:], mult)

            if has_bias:
                bias_bc = const.tile([P, FEAT], f32, tag="biasbc")
                nc.sync.dma_start(
                    out=bias_bc[:], in_=bias_d[0:1, :].broadcast_to([P, FEAT])
                )

            def jd(d, n):  # shifted identity J_d [128, n]
                return idw[:, d + 1 : d + 1 + n]

            def sv(d, c):  # per-partition band scalar for diag d, chunk c
                return uvw[:, d * NB + c : d * NB + c + 1]

            # E_c[p, q] = eff[R+p, C+q]: diag d=p-q==delta-1 -> w[R+p],
            # ==delta -> v[R+p], ==delta+1 -> u[R+p]
            eblocks = []
            for c, R, K, C, N, delta in chunks:
                E = const.tile([P, P + 1], f32, tag=f"E{c}", name=f"E{c}")
                nc.vector.tensor_scalar(
                    E[:, 0:N], jd(delta - 1, N), sv(2, c), None, mult
                )
                nc.vector.scalar_tensor_tensor(
                    E[:, 0:N], jd(delta, N), sv(1, c), E[:, 0:N], mult, add
                )
                nc.vector.scalar_tensor_tensor(
                    E[:, 0:N], jd(delta + 1, N), sv(0, c), E[:, 0:N], mult, add
                )
                eblocks.append(E)

            # whole xT shard in SBUF once, as 33 overlapping row-slabs
            # [K, 1024] (~132KB/partition); reused by all 8 m-blocks
            X = xp.tile([P, n_chunks, TOK_PER_CORE], f32, tag="X")
            for c, R, K, C, N, delta in chunks:
                nc.sync.dma_start(out=X[0:K, c, :], in_=xT_d[R : R + K, :])

            ablate = os.environ.get("KERNEL_ABLATE", "")
            # chunks grouped 4-per-PSUM-bank: the first matmul in a group
            # arms the 2KB bank (start=True); later matmuls overwrite their
            # own still-pending columns; one copy evicts the whole group.
            GRP = int(os.environ.get("KERNEL_GRP", "1"))
            groups = [chunks[i : i + GRP] for i in range(0, n_chunks, GRP)]
            # out DMA piece boundaries, in units of groups
            per = int(os.environ.get("KERNEL_PIECE_GROUPS", "0")) or max(1, len(chunks) // (4 * GRP))
            cmode = os.environ.get("KERNEL_COPY", "a")
            for m in range(n_m):
                t0 = m * P
                out_m = op.tile([P, FEAT], f32, tag="out")
                if ablate:
                    nc.vector.memset(out_m[:, 0:1], 0.0)
                col0 = 0
                for g, grp in enumerate(groups):
                    gC = grp[0][3]  # first col of group
                    gH = grp[-1][3] + grp[-1][4]  # end col
                    if "nomm" not in ablate:
                        pt = pp.tile([P, 512], f32, tag="ps", name=f"ps_{m}_{g}")
                        for j, (c, R, K, C, N, delta) in enumerate(grp):
                            nc.tensor.matmul(
                                pt[0:P, C - gC : C - gC + N],
                                X[0:K, c, t0 : t0 + P],
                                eblocks[c][0:K, 0:N],
                                start=(j == 0),
                                stop=(j == len(grp) - 1),
                            )
                        if "nocopy" not in ablate:
                            eng = [ch for ch in cmode][g % len(cmode)]
                            if eng == "v":
                                nc.vector.tensor_copy(
                                    out_m[:, gC:gH], pt[:, 0 : gH - gC]
                                )
                            elif eng == "s":
                                nc.scalar.copy(
                                    out_m[:, gC:gH], pt[:, 0 : gH - gC]
                                )
                            else:
                                nc.any.tensor_copy(
                                    out_m[:, gC:gH], pt[:, 0 : gH - gC]
                                )
                    if g % per == per - 1 or g == len(groups) - 1:
                        if has_bias:
                            nc.gpsimd.tensor_tensor(
                                out_m[:, col0:gH],
                                out_m[:, col0:gH],
                                bias_bc[:, col0:gH],
                                add,
                            )
                        nc.sync.dma_start(
                            out=y_d[t0 : t0 + P, col0:gH],
                            in_=out_m[:, col0:gH],
                        )
                        col0 = gH

    nc.compile()
    return nc


def _pe_chunks_u():
    """Uniform 126-col chunks for the fp16 PE kernel.

    Chunk c produces output cols [126c, 126c+126) from x rows
    [126c-1, 126c+127) (the tridiagonal band needs rows col-1..col+1).
    Uniform widths let adjacent chunks share merged DMA transfers.
    Returns list of (c, R, K, C, N, delta) with delta = C - R.
    """
    out = [(0, 0, 127, 0, 126, 0)]
    for c in range(1, 32):
        out.append((c, 126 * c - 1, 128, 126 * c, 126, 1))
    out.append((32, 4031, 65, 4032, 64, 1))
    return out


def _build_banded_pe16_program(has_bias: bool):
    """v4: fp16 I/O banded matmul, E stationary / X moving, yT output.

    fp16 x in, fp16 y out (host casts to/from fp32; the 2e-2 rel-err
    budget dwarfs fp16's ~5e-4) halves HBM traffic — the roofline —
    and makes matmuls 1 cycle/row instead of 4.

    Each E block is the PE-stationary operand and 512-token X slices
    the moving one: 2 matmuls per chunk (66 total) instead of the fp32
    kernel's 264, which had saturated the PE sequencer.  Outputs land
    transposed ([cols, tokens]) in PSUM, are cast-copied (alternating
    DVE/ACT) into one big fp16 yT tile, and leave as 4-chunk merged
    DMA stores from the Pool queue (SP = pure load queue, ACT = band
    loads + half the copies).  Uniform 126-col chunks make the merged
    load/store access patterns regular; x rows overlap by 2 between
    chunks (0.35us extra read traffic — accepted).
    """
    import concourse.bass as bass  # noqa: F401
    import concourse.mybir as mybir
    import concourse.tile as tile
    from concourse import bacc

    f32 = mybir.dt.float32
    f16 = mybir.dt.float16
    mult = mybir.AluOpType.mult
    add = mybir.AluOpType.add

    nc = bacc.Bacc("TRN2", target_bir_lowering=False, debug=False)

    chunks = _pe_chunks_u()
    NB = len(chunks)  # 33
    HTOK = TOK_PER_CORE // 2  # 512-token moving blocks
    TOK = TOK_PER_CORE

    xT_d = nc.dram_tensor("xT", [FEAT, TOK], f16, kind="ExternalInput").ap()
    cb_d = nc.dram_tensor("cbT", [P, 3 * NB], f32, kind="ExternalInput").ap()
    nb_d = nc.dram_tensor("nbT", [P, 3 * NB], f32, kind="ExternalInput").ap()
    wb_d = nc.dram_tensor("wbT", [P, 3 * NB], f32, kind="ExternalInput").ap()
    if has_bias:
        bias_d = nc.dram_tensor("bias", [FEAT, 1], f32, kind="ExternalInput").ap()
    y_d = nc.dram_tensor("y", [FEAT, TOK], f16, kind="ExternalOutput").ap()

    with tile.TileContext(nc) as tc:
        with (
            tc.tile_pool(name="const", bufs=1) as const,
            tc.tile_pool(name="xp", bufs=1) as xp,
            tc.tile_pool(name="op", bufs=1) as op,
            tc.tile_pool(name="pp", bufs=8, space="PSUM") as pp,
        ):
            # IDW[p, q] = 1 iff p == q-1 (fp16); IDW[:, d+1 : d+1+N] is the
            # shifted identity J_d[p, q] = [p == q+d] for d in -1..2
            idw = const.tile([P, P + 2], f16, tag="idw")
            nc.gpsimd.memset(idw[:], 0.0)
            nc.gpsimd.affine_select(
                out=idw[:],
                in_=idw[:],
                compare_op=mybir.AluOpType.not_equal,
                fill=1.0,
                base=1,
                pattern=[[-1, P + 2]],
                channel_multiplier=1,
            )

            # band loads go out on the ACT queue so the SP queue starts the
            # big X streams immediately
            cb_sb = const.tile([P, 3 * NB], f32, tag="cb")
            nb_sb = const.tile([P, 3 * NB], f32, tag="nb")
            wb_sb = const.tile([P, 3 * NB], f32, tag="wb")
            nc.scalar.dma_start(out=cb_sb[:], in_=cb_d[:])
            nc.scalar.dma_start(out=nb_sb[:], in_=nb_d[:])
            nc.scalar.dma_start(out=wb_sb[:], in_=wb_d[:])
            uvw = const.tile([P, 3 * NB], f32, tag="uvw")
            nc.vector.tensor_tensor(uvw[:], cb_sb[:], nb_sb[:], mult)
            nc.vector.tensor_tensor(uvw[:], uvw[:], wb_sb[:], mult)

            def jd(d, n):  # shifted identity J_d [128, n]
                return idw[:, d + 1 : d + 1 + n]

            def sv(d, c):  # per-partition band scalar for diag d, chunk c
                return uvw[:, d * NB + c : d * NB + c + 1]

            # E_c[p, q] = eff[R+p, C+q]: diag d=p-q==delta-1 -> w[R+p],
            # ==delta -> v[R+p], ==delta+1 -> u[R+p]
            eblocks = []
            for c, R, K, C, N, delta in chunks:
                E = const.tile([P, P + 1], f16, tag=f"E{c}", name=f"E{c}")
                nc.vector.tensor_scalar(
                    E[:, 0:N], jd(delta - 1, N), sv(2, c), None, mult
                )
                nc.vector.scalar_tensor_tensor(
                    E[:, 0:N], jd(delta, N), sv(1, c), E[:, 0:N], mult, add
                )
                nc.vector.scalar_tensor_tensor(
                    E[:, 0:N], jd(delta + 1, N), sv(0, c), E[:, 0:N], mult, add
                )
                eblocks.append(E)

            # fp16 xT shard in SBUF as 33 overlapping row-slabs; merged
            # 4-chunk loads (custom in-APs re-read the 2 overlap rows)
            X = xp.tile([P, NB, TOK], f16, tag="X")
            nc.sync.dma_start(out=X[0:127, 0, :], in_=xT_d[0:127, :])
            for c0 in range(1, 32, 4):
                G = min(4, 32 - c0)
                src = bass.AP(
                    xT_d.tensor,
                    (126 * c0 - 1) * TOK,
                    [[TOK, P], [126 * TOK, G], [1, TOK]],
                )
                nc.sync.dma_start(out=X[0:P, c0 : c0 + G, :], in_=src)
            nc.sync.dma_start(out=X[0:65, 32, :], in_=xT_d[4031:4096, :])

            if has_bias:
                # bias values for chunk c's output cols, one per partition
                bias_sb = const.tile([P, NB], f32, tag="biassb")
                for c, R, K, C, N, delta in chunks:
                    nc.scalar.dma_start(
                        out=bias_sb[0:N, c : c + 1], in_=bias_d[C : C + N, :]
                    )

            YT = op.tile([P, NB, TOK], f16, tag="YT")
            ncopy = 0
            for c, R, K, C, N, delta in chunks:
                for h in range(2):
                    pt = pp.tile([P, 512], f32, tag="ps", name=f"ps_{c}_{h}")
                    nc.tensor.matmul(
                        pt[0:N, 0:HTOK],
                        eblocks[c][0:K, 0:N],
                        X[0:K, c, h * HTOK : (h + 1) * HTOK],
                        start=True,
                        stop=True,
                    )
                    dst = YT[0:N, c, h * HTOK : (h + 1) * HTOK]
                    if has_bias:
                        # fold bias into the PSUM->SBUF cast copy
                        nc.vector.tensor_scalar(
                            dst,
                            pt[0:N, 0:HTOK],
                            bias_sb[0:N, c : c + 1],
                            None,
                            add,
                        )
                    elif ncopy % 2 == 0:
                        nc.vector.tensor_copy(dst, pt[0:N, 0:HTOK])
                    else:
                        nc.scalar.copy(dst, pt[0:N, 0:HTOK])
                    ncopy += 1
                # merged 4-chunk stores from the Pool queue once the last
                # chunk of each group is copied
                if c % 4 == 3:
                    c0 = c - 3
                    dstp = bass.AP(
                        y_d.tensor,
                        126 * c0 * TOK,
                        [[TOK, 126], [126 * TOK, 4], [1, TOK]],
                    )
                    nc.gpsimd.dma_start(out=dstp, in_=YT[0:126, c0 : c0 + 4, :])
            nc.gpsimd.dma_start(out=y_d[4032:4096, :], in_=YT[0:64, 32, :])

    nc.compile()
    return nc


def _gather_bands_pe_u(connections, nearest_neighbors, weight):
    """Row-diagonal bands for the uniform-chunk fp16 PE kernel.

    Same band semantics as _gather_bands_pe (u[i] = factor of
    eff[i, i-1], v[i] = eff[i, i], w[i] = eff[i, i+1]; products on
    device), but packed per _pe_chunks_u's row windows: column d*NB + c
    holds band_d[R_c + p] at partition p.
    """
    chunks = _pe_chunks_u()
    NB = len(chunks)
    z1 = np.zeros(1, np.float32)

    def pack(u, v, w):
        out = np.zeros((P, 3 * NB), np.float32)
        for d, band in enumerate((u, v, w)):
            for c, R, K, C, N, delta in chunks:
                out[:K, d * NB + c] = band[R : R + K]
        return out

    def bands(m, transposed):
        up = np.ascontiguousarray(np.diagonal(m, 1)).astype(np.float32, copy=False)
        mid = np.ascontiguousarray(np.diagonal(m, 0)).astype(np.float32, copy=False)
        dn = np.ascontiguousarray(np.diagonal(m, -1)).astype(np.float32, copy=False)
        if transposed:  # weight[out, in]: need w[i-1,i], w[i,i], w[i+1,i]
            u = np.concatenate([z1, up])  # weight[i-1, i] = diag(w,+1)[i-1]
            w = np.concatenate([dn, z1])  # weight[i+1, i] = diag(w,-1)[i]
        else:  # conn/nn [i, j]: need m[i, i-1], m[i, i], m[i, i+1]
            u = np.concatenate([z1, dn])  # m[i, i-1] = diag(m,-1)[i-1]
            w = np.concatenate([up, z1])  # m[i, i+1] = diag(m,+1)[i]
        return pack(u, mid, w)

    return (
        bands(connections, False),
        bands(nearest_neighbors, False),
        bands(weight, True),
    )


def _gather_bands_pe(connections, nearest_neighbors, weight):
    """Row-diagonal bands for the PE kernel, packed [128, 3*NB].

    u[i] = factor of eff[i, i-1], v[i] = eff[i, i], w[i] = eff[i, i+1]
    (per input matrix; products are computed on device).  Column d*NB + c
    holds band_d[126c + p] at partition p, zero-padded past index 4095.
    """
    NB = len(_pe_chunks())
    z1 = np.zeros(1, np.float32)

    def pack(u, v, w):
        out = np.zeros((P, 3 * NB), np.float32)
        for d, band in enumerate((u, v, w)):
            for c in range(NB):
                lo = 126 * c
                n = min(P, len(band) - lo)
                if n > 0:
                    out[:n, d * NB + c] = band[lo : lo + n]
        return out

    def bands(m, transposed):
        up = np.ascontiguousarray(np.diagonal(m, 1)).astype(np.float32, copy=False)
        mid = np.ascontiguousarray(np.diagonal(m, 0)).astype(np.float32, copy=False)
        dn = np.ascontiguousarray(np.diagonal(m, -1)).astype(np.float32, copy=False)
        if transposed:  # weight[out, in]: need w[i-1,i], w[i,i], w[i+1,i]
            u = np.concatenate([z1, up])  # weight[i-1, i] = diag(w,+1)[i-1]
            w = np.concatenate([dn, z1])  # weight[i+1, i] = diag(w,-1)[i]
        else:  # conn/nn [i, j]: need m[i, i-1], m[i, i], m[i, i+1]
            u = np.concatenate([z1, dn])  # m[i, i-1] = diag(m,-1)[i-1]
            w = np.concatenate([up, z1])  # m[i, i+1] = diag(m,+1)[i]
        return pack(u, mid, w)

    return (
        bands(connections, False),
        bands(nearest_neighbors, False),
        bands(weight, True),
    )


def _gather_bands(connections, nearest_neighbors, weight):
    """Pure indexing: extract the 3 relevant diagonals of each operand.

    Row 0 (A): entries for eff[j-1, j]  -> conn[j-1,j], nn[j-1,j], w[j,j-1]
    Row 1 (B): entries for eff[j, j]    -> conn[j,j],   nn[j,j],   w[j,j]
    Row 2 (C): entries for eff[j+1, j]  -> conn[j+1,j], nn[j+1,j], w[j,j+1]
    Out-of-range slots are zero-padded.
    """
    z1 = np.zeros(1, np.float32)

    def band3(m, transposed):
        # For conn/nn (indexed [i, j] = [row, out-col]):
        #   A[j] = m[j-1, j] = diag(m, +1) shifted;  B = diag(m, 0);
        #   C[j] = m[j+1, j] = diag(m, -1)
        # For weight (indexed [out, in] -> we need w[j, j-1], w[j,j], w[j,j+1]):
        #   A[j] = w[j, j-1] = diag(w, -1) shifted;  B = diag(w, 0);
        #   C[j] = w[j, j+1] = diag(w, +1)
        up = np.ascontiguousarray(np.diagonal(m, 1)).astype(np.float32, copy=False)
        mid = np.ascontiguousarray(np.diagonal(m, 0)).astype(np.float32, copy=False)
        dn = np.ascontiguousarray(np.diagonal(m, -1)).astype(np.float32, copy=False)
        if transposed:  # weight
            a = np.concatenate([z1, dn])
            c = np.concatenate([up, z1])
        else:  # conn / nn
            a = np.concatenate([z1, up])
            c = np.concatenate([dn, z1])
        return np.ascontiguousarray(np.stack([a, mid, c]))

    return (
        band3(connections, False),
        band3(nearest_neighbors, False),
        band3(weight, True),
    )


def kernel(x, connections, nearest_neighbors, weight, bias):
    global LAST_RESULTS
    x = np.asarray(x, dtype=np.float32)
    connections = np.asarray(connections, dtype=np.float32)
    nearest_neighbors = np.asarray(nearest_neighbors, dtype=np.float32)
    weight = np.asarray(weight, dtype=np.float32)
    bias = np.asarray(bias, dtype=np.float32)

    # Safety net: the device kernel assumes nearest_neighbors is zero
    # outside the tridiagonal band (true for this problem by construction).
    i = np.arange(FEAT)
    off_band = np.abs(i[:, None] - i[None, :]) > 1
    if np.any(nearest_neighbors[off_band] != 0.0):
        eff = connections * nearest_neighbors * weight.T
        return (x @ eff + bias).astype(np.float32)

    from concourse.bass_utils import run_bass_kernel_spmd

    has_bias = bool(np.any(bias != 0.0))
    impl = os.environ.get("KERNEL_IMPL", "pe16")
    key = (impl, has_bias)
    if key not in _cached:
        builder = {
            "pe16": _build_banded_pe16_program,
            "pe": _build_banded_pe_program,
            "banded": _build_banded_program,
        }[impl]
        _cached[key] = builder(has_bias)
    nc = _cached[key]

    in_maps = []
    if impl == "pe16":
        cb, nb, wb = _gather_bands_pe_u(connections, nearest_neighbors, weight)
        xT = x.astype(np.float16).T  # [FEAT, BATCH] view of the fp16 cast
        for c in range(N_CORES):
            m = {
                "xT": np.ascontiguousarray(
                    xT[:, c * TOK_PER_CORE : (c + 1) * TOK_PER_CORE]
                ),
                "cbT": cb,
                "nbT": nb,
                "wbT": wb,
            }
            if has_bias:
                m["bias"] = np.ascontiguousarray(bias.reshape(FEAT, 1))
            in_maps.append(m)
    elif impl == "pe":
        cb, nb, wb = _gather_bands_pe(connections, nearest_neighbors, weight)
        xT = np.ascontiguousarray(x.T)
        for c in range(N_CORES):
            m = {
                "xT": np.ascontiguousarray(
                    xT[:, c * TOK_PER_CORE : (c + 1) * TOK_PER_CORE]
                ),
                "cbT": cb,
                "nbT": nb,
                "wbT": wb,
            }
            if has_bias:
                m["bias"] = np.ascontiguousarray(bias.reshape(1, FEAT))
            in_maps.append(m)
    else:
        cb, nb, wb = _gather_bands(connections, nearest_neighbors, weight)
        for c in range(N_CORES):
            m = {
                "x": np.ascontiguousarray(
                    x[c * TOK_PER_CORE : (c + 1) * TOK_PER_CORE, :]
                ),
                "conn_band": cb,
                "nn_band": nb,
                "w_band": wb,
            }
            if has_bias:
                m["bias"] = np.ascontiguousarray(bias.reshape(1, FEAT))
            in_maps.append(m)

    trace = bool(int(os.environ.get("KERNEL_TRACE", "0")))
    res = run_bass_kernel_spmd(
        nc, in_maps, core_ids=list(range(N_CORES)), trace=trace
    )
    LAST_RESULTS = res

    out = np.empty((BATCH, FEAT), dtype=np.float32)
    for c in range(N_CORES):
        yc = np.asarray(res.results[c]["y"])
        if yc.shape == (FEAT, TOK_PER_CORE):  # transposed (pe16) layout
            yc = yc.T
        out[c * TOK_PER_CORE : (c + 1) * TOK_PER_CORE, :] = yc.astype(
            np.float32
        )
    return out



# revision 25
# speedup vs baseline: 2.0490x; 1.0955x over previous
"""Trainium2 Bass kernel for NearestNeighborSparseLayer.

Reference computation:
    eff = connections * nearest_neighbors * weight.T   # [in, out]
    out = x @ eff + bias                                # [8192, 4096]

`nearest_neighbors` is a tridiagonal mask (|i-j| <= 1), so `eff` has at
most 3 nonzero diagonals and the matmul collapses to a banded (3-tap)
elementwise operation along the feature axis:

    out[t, j] = x[t, j-1]*cA[j] + x[t, j]*cB[j] + x[t, j+1]*cC[j] + bias[j]

where cA[j] = eff[j-1, j], cB[j] = eff[j, j], cC[j] = eff[j+1, j].

Strategy: data-parallel over the 8192 token rows across 8 NeuronCores
(1024 rows/core).  The host only slices/reformats data (sharding, band
gathering via np.diagonal, replication); all arithmetic — the
connections*nearest_neighbors*weight products and the banded multiply-
accumulate — runs on-device.

If `nearest_neighbors` is NOT band-limited (never the case for this
problem's input generator, which builds a tridiagonal mask), we fall
back to a plain numpy evaluation for correctness.
"""

import os

import numpy as np

BATCH = 8192
FEAT = 4096
N_CORES = 8
TOK_PER_CORE = BATCH // N_CORES  # 1024
P = 128  # partitions

LAST_RESULTS = None  # BassKernelResults from the most recent run (for test.py)

_cached = {}  # (has_bias,) -> compiled Bass program


def _build_banded_program(has_bias: bool):
    import concourse.bass as bass  # noqa: F401
    import concourse.mybir as mybir
    import concourse.tile as tile
    from concourse import bacc

    f32 = mybir.dt.float32
    mult = mybir.AluOpType.mult
    add = mybir.AluOpType.add

    nc = bacc.Bacc("TRN2", target_bir_lowering=False, debug=False)

    x_d = nc.dram_tensor("x", [TOK_PER_CORE, FEAT], f32, kind="ExternalInput").ap()
    cb_d = nc.dram_tensor("conn_band", [3, FEAT], f32, kind="ExternalInput").ap()
    nb_d = nc.dram_tensor("nn_band", [3, FEAT], f32, kind="ExternalInput").ap()
    wb_d = nc.dram_tensor("w_band", [3, FEAT], f32, kind="ExternalInput").ap()
    if has_bias:
        bias_d = nc.dram_tensor("bias", [1, FEAT], f32, kind="ExternalInput").ap()
    y_d = nc.dram_tensor("y", [TOK_PER_CORE, FEAT], f32, kind="ExternalOutput").ap()

    n_tiles = TOK_PER_CORE // P  # 8

    # bands live as [96, 128] tiles (3*4096 elements spread over 96
    # partitions) so they cost 512B/partition instead of 16KB/partition
    bp, bf = 96, 128

    with tile.TileContext(nc) as tc:
        with (
            tc.tile_pool(name="const", bufs=1) as const,
            tc.tile_pool(name="xp", bufs=2) as xp,
            tc.tile_pool(name="tp", bufs=2) as tp,
            tc.tile_pool(name="dram", bufs=1, space="DRAM") as dram,
        ):
            # --- one-time: compute banded coefficients on device ---
            cb_sb = const.tile([bp, bf], f32, tag="cb")
            nb_sb = const.tile([bp, bf], f32, tag="nb")
            wb_sb = const.tile([bp, bf], f32, tag="wb")
            r96 = lambda ap: ap.rearrange("a (b c) -> (a b) c", c=bf)
            nc.sync.dma_start(out=cb_sb[:], in_=r96(cb_d))
            nc.sync.dma_start(out=nb_sb[:], in_=r96(nb_d))
            nc.sync.dma_start(out=wb_sb[:], in_=r96(wb_d))
            coef = const.tile([bp, bf], f32, tag="coef")
            nc.vector.tensor_tensor(coef[:], cb_sb[:], nb_sb[:], mult)
            nc.vector.tensor_tensor(coef[:], coef[:], wb_sb[:], mult)

            # round-trip through DRAM so we can broadcast each row across
            # all 128 partitions with a step-0 DMA read
            coef_dram = dram.tile([3, FEAT], f32, tag="coefd")
            nc.sync.dma_start(out=r96(coef_dram[:]), in_=coef[:])

            A = const.tile([P, FEAT], f32, tag="A")
            B = const.tile([P, FEAT], f32, tag="B")
            C = const.tile([P, FEAT], f32, tag="C")
            nc.sync.dma_start(out=A[:], in_=coef_dram[0:1, :].broadcast_to([P, FEAT]))
            nc.sync.dma_start(out=B[:], in_=coef_dram[1:2, :].broadcast_to([P, FEAT]))
            nc.sync.dma_start(out=C[:], in_=coef_dram[2:3, :].broadcast_to([P, FEAT]))
            if has_bias:
                BI = const.tile([P, FEAT], f32, tag="BI")
                nc.sync.dma_start(
                    out=BI[:], in_=bias_d[0:1, :].broadcast_to([P, FEAT])
                )

            # --- main loop: banded 3-tap multiply-accumulate ---
            for i in range(n_tiles):
                r0 = i * P
                xt = xp.tile([P, FEAT + 2], f32, tag="x")
                nc.vector.memset(xt[:, 0:1], 0.0)
                nc.vector.memset(xt[:, FEAT + 1 : FEAT + 2], 0.0)
                nc.sync.dma_start(out=xt[:, 1 : FEAT + 1], in_=x_d[r0 : r0 + P, :])

                t_a = tp.tile([P, FEAT], f32, tag="ta")
                t_b = tp.tile([P, FEAT], f32, tag="tb")
                t_c = tp.tile([P, FEAT], f32, tag="tc")

                # x[t, j-1] * cA[j]
                nc.vector.tensor_tensor(t_a[:], xt[:, 0:FEAT], A[:], mult)
                # x[t, j+1] * cC[j]
                nc.vector.tensor_tensor(t_c[:], xt[:, 2 : FEAT + 2], C[:], mult)
                # x[t, j] * cB[j]   (gpsimd runs in parallel with DVE)
                nc.gpsimd.tensor_tensor(t_b[:], xt[:, 1 : FEAT + 1], B[:], mult)
                # t_a += t_c  (in-place: identical in/out APs are safe for
                # elementwise streaming ops)
                nc.vector.tensor_tensor(t_a[:], t_a[:], t_c[:], add)
                if has_bias:
                    nc.gpsimd.tensor_tensor(t_b[:], t_b[:], BI[:], add)
                nc.gpsimd.tensor_tensor(t_b[:], t_a[:], t_b[:], add)

                nc.sync.dma_start(out=y_d[r0 : r0 + P, :], in_=t_b[:])

    nc.compile()
    return nc


def _pe_chunks():
    """Non-overlapping column chunks for the PE-banded kernel.

    Chunk c produces output columns [C_c, C_c + N_c) from input rows
    [R_c, R_c + K_c), where the 3-diagonal band makes each column depend on
    rows col-1..col+1.  With R_c = 126*c the row windows fit in 128
    partitions and every output column is produced by exactly ONE matmul
    (no PSUM accumulation).  delta = C_c - R_c selects which diagonals of
    the rhs block are populated.

    Returns list of (c, R, K, C, N, delta).
    """
    chunks = []
    c = 0
    col = 0
    while col < FEAT:
        R = 126 * c
        K = min(P, FEAT - R)
        delta = col - R  # 0 for chunk 0, 1 afterwards
        max_col = FEAT - 1 if R + K >= FEAT else R + K - 2
        N = max_col - col + 1
        chunks.append((c, R, K, col, N, delta))
        col += N
        c += 1
    return chunks


def _build_banded_pe_program(has_bias: bool):
    """v2: banded matmul on the tensor engine, non-overlapping chunks.

    For each chunk (R, K, C, N, delta):
        out[tokens, C:C+N] = xT[R:R+K, tokens].T @ E_c[0:K, 0:N]
    where E_c is the dense banded block of eff rows R..R+K-1 x cols
    C..C+N-1, built on device from the gathered diagonals.  Every output
    column is produced by exactly one matmul (start=stop=True), so no
    PSUM accumulation semantics are needed.
    """
    import concourse.bass as bass  # noqa: F401
    import concourse.mybir as mybir
    import concourse.tile as tile
    from concourse import bacc

    f32 = mybir.dt.float32
    mult = mybir.AluOpType.mult
    add = mybir.AluOpType.add

    nc = bacc.Bacc("TRN2", target_bir_lowering=False, debug=False)

    chunks = _pe_chunks()
    n_chunks = len(chunks)  # 33
    n_m = TOK_PER_CORE // P  # 8
    NB = n_chunks  # band columns per diagonal

    xT_d = nc.dram_tensor("xT", [FEAT, TOK_PER_CORE], f32, kind="ExternalInput").ap()
    # bands packed [128, 3*NB]: col d*NB + c holds band_d[126c + p] at
    # partition p (d: 0=u sub, 1=v main, 2=w super diag of eff's rows)
    cb_d = nc.dram_tensor("cbT", [P, 3 * NB], f32, kind="ExternalInput").ap()
    nb_d = nc.dram_tensor("nbT", [P, 3 * NB], f32, kind="ExternalInput").ap()
    wb_d = nc.dram_tensor("wbT", [P, 3 * NB], f32, kind="ExternalInput").ap()
    if has_bias:
        bias_d = nc.dram_tensor("bias", [1, FEAT], f32, kind="ExternalInput").ap()
    y_d = nc.dram_tensor("y", [TOK_PER_CORE, FEAT], f32, kind="ExternalOutput").ap()

    with tile.TileContext(nc) as tc:
        with (
            tc.tile_pool(name="const", bufs=1) as const,
            tc.tile_pool(name="xp", bufs=1) as xp,
            tc.tile_pool(name="op", bufs=int(os.environ.get("KERNEL_OPBUFS", "2"))) as op,
            tc.tile_pool(name="pp", bufs=8, space="PSUM") as pp,
        ):
            # IDW[p, q] = 1 iff p == q-1; slicing IDW[:, d+1 : d+1+N] gives
            # the shifted identity J_d[p, q] = [p == q+d] for d in -1..2
            idw = const.tile([P, P + 2], f32, tag="idw")
            nc.gpsimd.memset(idw[:], 0.0)
            nc.gpsimd.affine_select(
                out=idw[:],
                in_=idw[:],
                compare_op=mybir.AluOpType.not_equal,
                fill=1.0,
                base=1,
                # fill where (p - q + 1) == 0, i.e. at q = p+1
                pattern=[[-1, P + 2]],
                channel_multiplier=1,
            )

            cb_sb = const.tile([P, 3 * NB], f32, tag="cb")
            nb_sb = const.tile([P, 3 * NB], f32, tag="nb")
            wb_sb = const.tile([P, 3 * NB], f32, tag="wb")
            nc.sync.dma_start(out=cb_sb[:], in_=cb_d[:])
            nc.sync.dma_start(out=nb_sb[:], in_=nb_d[:])
            nc.sync.dma_start(out=wb_sb[:], in_=wb_d[:])
            uvw = const.tile([P, 3 * NB], f32, tag="uvw")
            nc.vector.tensor_tensor(uvw[:], cb_sb[:], nb_sb[:], mult)
            nc.vector.tensor_tensor(uvw[:], uvw[:], wb_sb[:], mult)

            if has_bias:
                bias_bc = const.tile([P, FEAT], f32, tag="biasbc")
                nc.sync.dma_start(
                    out=bias_bc[:], in_=bias_d[0:1, :].broadcast_to([P, FEAT])
                )

            def jd(d, n):  # shifted identity J_d [128, n]
                return idw[:, d + 1 : d + 1 + n]

            def sv(d, c):  # per-partition band scalar for diag d, chunk c
                return uvw[:, d * NB + c : d * NB + c + 1]

            # E_c[p, q] = eff[R+p, C+q]: diag d=p-q==delta-1 -> w[R+p],
            # ==delta -> v[R+p], ==delta+1 -> u[R+p]
            eblocks = []
            for c, R, K, C, N, delta in chunks:
                E = const.tile([P, P + 1], f32, tag=f"E{c}", name=f"E{c}")
                nc.vector.tensor_scalar(
                    E[:, 0:N], jd(delta - 1, N), sv(2, c), None, mult
                )
                nc.vector.scalar_tensor_tensor(
                    E[:, 0:N], jd(delta, N), sv(1, c), E[:, 0:N], mult, add
                )
                nc.vector.scalar_tensor_tensor(
                    E[:, 0:N], jd(delta + 1, N), sv(0, c), E[:, 0:N], mult, add
                )
                eblocks.append(E)

            # whole xT shard in SBUF once, as 33 overlapping row-slabs
            # [K, 1024] (~132KB/partition); reused by all 8 m-blocks
            X = xp.tile([P, n_chunks, TOK_PER_CORE], f32, tag="X")
            for c, R, K, C, N, delta in chunks:
                nc.sync.dma_start(out=X[0:K, c, :], in_=xT_d[R : R + K, :])

            ablate = os.environ.get("KERNEL_ABLATE", "")
            # chunks grouped 4-per-PSUM-bank: the first matmul in a group
            # arms the 2KB bank (start=True); later matmuls overwrite their
            # own still-pending columns; one copy evicts the whole group.
            GRP = int(os.environ.get("KERNEL_GRP", "1"))
            groups = [chunks[i : i + GRP] for i in range(0, n_chunks, GRP)]
            # out DMA piece boundaries, in units of groups
            per = int(os.environ.get("KERNEL_PIECE_GROUPS", "0")) or max(1, len(chunks) // (4 * GRP))
            cmode = os.environ.get("KERNEL_COPY", "a")
            for m in range(n_m):
                t0 = m * P
                out_m = op.tile([P, FEAT], f32, tag="out")
                if ablate:
                    nc.vector.memset(out_m[:, 0:1], 0.0)
                col0 = 0
                for g, grp in enumerate(groups):
                    gC = grp[0][3]  # first col of group
                    gH = grp[-1][3] + grp[-1][4]  # end col
                    if "nomm" not in ablate:
                        pt = pp.tile([P, 512], f32, tag="ps", name=f"ps_{m}_{g}")
                        for j, (c, R, K, C, N, delta) in enumerate(grp):
                            nc.tensor.matmul(
                                pt[0:P, C - gC : C - gC + N],
                                X[0:K, c, t0 : t0 + P],
                                eblocks[c][0:K, 0:N],
                                start=(j == 0),
                                stop=(j == len(grp) - 1),
                            )
                        if "nocopy" not in ablate:
                            eng = [ch for ch in cmode][g % len(cmode)]
                            if eng == "v":
                                nc.vector.tensor_copy(
                                    out_m[:, gC:gH], pt[:, 0 : gH - gC]
                                )
                            elif eng == "s":
                                nc.scalar.copy(
                                    out_m[:, gC:gH], pt[:, 0 : gH - gC]
                                )
                            else:
                                nc.any.tensor_copy(
                                    out_m[:, gC:gH], pt[:, 0 : gH - gC]
                                )
                    if g % per == per - 1 or g == len(groups) - 1:
                        if has_bias:
                            nc.gpsimd.tensor_tensor(
                                out_m[:, col0:gH],
                                out_m[:, col0:gH],
                                bias_bc[:, col0:gH],
                                add,
                            )
                        nc.sync.dma_start(
                            out=y_d[t0 : t0 + P, col0:gH],
                            in_=out_m[:, col0:gH],
                        )
                        col0 = gH

    nc.compile()
    return nc


def _pe_chunks_u():
    """Uniform 126-col chunks for the fp16 PE kernel.

    Chunk c produces output cols [126c, 126c+126) from x rows
    [126c-1, 126c+127) (the tridiagonal band needs rows col-1..col+1).
    Uniform widths let adjacent chunks share merged DMA transfers.
    Returns list of (c, R, K, C, N, delta) with delta = C - R.
    """
    out = [(0, 0, 127, 0, 126, 0)]
    for c in range(1, 32):
        out.append((c, 126 * c - 1, 128, 126 * c, 126, 1))
    out.append((32, 4031, 65, 4032, 64, 1))
    return out


def _build_banded_pe16_program(has_bias: bool):
    """v4: fp16 I/O banded matmul, E stationary / X moving, yT output.

    fp16 x in, fp16 y out (host casts to/from fp32; the 2e-2 rel-err
    budget dwarfs fp16's ~5e-4) halves HBM traffic — the roofline —
    and makes matmuls 1 cycle/row instead of 4.

    Each E block is the PE-stationary operand and 512-token X slices
    the moving one: 2 matmuls per chunk (66 total) instead of the fp32
    kernel's 264, which had saturated the PE sequencer.  Outputs land
    transposed ([cols, tokens]) in PSUM, are cast-copied (alternating
    DVE/ACT) into one big fp16 yT tile, and leave as 4-chunk merged
    DMA stores from the Pool queue (SP = pure load queue, ACT = band
    loads + half the copies).  Uniform 126-col chunks make the merged
    load/store access patterns regular; x rows overlap by 2 between
    chunks (0.35us extra read traffic — accepted).
    """
    import concourse.bass as bass  # noqa: F401
    import concourse.mybir as mybir
    import concourse.tile as tile
    from concourse import bacc

    f32 = mybir.dt.float32
    f16 = mybir.dt.float16
    mult = mybir.AluOpType.mult
    add = mybir.AluOpType.add

    nc = bacc.Bacc("TRN2", target_bir_lowering=False, debug=False)

    chunks = _pe_chunks_u()
    NB = len(chunks)  # 33
    HTOK = TOK_PER_CORE // 2  # 512-token moving blocks
    TOK = TOK_PER_CORE

    xT_d = nc.dram_tensor("xT", [FEAT, TOK], f16, kind="ExternalInput").ap()
    # conn/nn/weight band triples packed into one fp16 tensor: one full-
    # speed DMA instead of three sub-512B (2x latency penalty) transfers
    bands_d = nc.dram_tensor("bands", [P, 9 * NB], f16, kind="ExternalInput").ap()
    if has_bias:
        bias_d = nc.dram_tensor("bias", [FEAT, 1], f32, kind="ExternalInput").ap()
    y_d = nc.dram_tensor("y", [FEAT, TOK], f16, kind="ExternalOutput").ap()

    with tile.TileContext(nc) as tc:
        with (
            tc.tile_pool(name="const", bufs=1) as const,
            tc.tile_pool(name="xp", bufs=1) as xp,
            tc.tile_pool(name="op", bufs=1) as op,
            tc.tile_pool(name="pp", bufs=8, space="PSUM") as pp,
        ):
            # IDW[p, q] = 1 iff p == q-1 (fp16); IDW[:, d+1 : d+1+N] is the
            # shifted identity J_d[p, q] = [p == q+d] for d in -1..2
            idw = const.tile([P, P + 2], f16, tag="idw")
            nc.gpsimd.memset(idw[:], 0.0)
            nc.gpsimd.affine_select(
                out=idw[:],
                in_=idw[:],
                compare_op=mybir.AluOpType.not_equal,
                fill=1.0,
                base=1,
                pattern=[[-1, P + 2]],
                channel_multiplier=1,
            )

            # band load goes out on the ACT queue so the SP queue starts the
            # big X streams immediately
            bsb = const.tile([P, 9 * NB], f16, tag="bands")
            nc.scalar.dma_start(out=bsb[:], in_=bands_d[:])
            uvw = const.tile([P, 3 * NB], f32, tag="uvw")
            nc.vector.tensor_tensor(
                uvw[:], bsb[:, 0 : 3 * NB], bsb[:, 3 * NB : 6 * NB], mult
            )
            nc.vector.tensor_tensor(
                uvw[:], uvw[:], bsb[:, 6 * NB : 9 * NB], mult
            )

            def jd(d, n):  # shifted identity J_d [128, n]
                return idw[:, d + 1 : d + 1 + n]

            def sv(d, c):  # per-partition band scalar for diag d, chunk c
                return uvw[:, d * NB + c : d * NB + c + 1]

            # E_c[p, q] = eff[R+p, C+q]: diag d=p-q==delta-1 -> w[R+p],
            # ==delta -> v[R+p], ==delta+1 -> u[R+p].
            # All on DVE (TensorScalarPtr is not a legal Pool opcode on
            # core_v3), but software-pipelined: the first NPRE chunks are
            # built upfront, the rest interleave one-per-iteration into the
            # main loop so the 99 build ops never sit between the copies
            # and the stores they feed.
            eblocks = [
                const.tile([P, P + 1], f16, tag=f"E{c}", name=f"E{c}")
                for c in range(NB)
            ]

            def build_e(c):
                _, R, K, C, N, delta = chunks[c]
                E = eblocks[c]
                nc.vector.tensor_scalar(
                    E[:, 0:N], jd(delta - 1, N), sv(2, c), None, mult
                )
                nc.vector.scalar_tensor_tensor(
                    E[:, 0:N], jd(delta, N), sv(1, c), E[:, 0:N], mult, add
                )
                nc.vector.scalar_tensor_tensor(
                    E[:, 0:N], jd(delta + 1, N), sv(0, c), E[:, 0:N], mult, add
                )

            NPRE = 16
            for c in range(NPRE):
                build_e(c)

            # fp16 xT shard in SBUF as 33 overlapping row-slabs; merged
            # 4-chunk loads (custom in-APs re-read the 2 overlap rows)
            X = xp.tile([P, NB, TOK], f16, tag="X")
            nc.sync.dma_start(out=X[0:127, 0, :], in_=xT_d[0:127, :])
            for c0 in range(1, 32, 4):
                G = min(4, 32 - c0)
                src = bass.AP(
                    xT_d.tensor,
                    (126 * c0 - 1) * TOK,
                    [[TOK, P], [126 * TOK, G], [1, TOK]],
                )
                nc.sync.dma_start(out=X[0:P, c0 : c0 + G, :], in_=src)
            nc.sync.dma_start(out=X[0:65, 32, :], in_=xT_d[4031:4096, :])

            if has_bias:
                # bias values for chunk c's output cols, one per partition
                bias_sb = const.tile([P, NB], f32, tag="biassb")
                for c, R, K, C, N, delta in chunks:
                    nc.scalar.dma_start(
                        out=bias_sb[0:N, c : c + 1], in_=bias_d[C : C + N, :]
                    )

            YT = op.tile([P, NB, TOK], f16, tag="YT")
            ncopy = 0
            for c, R, K, C, N, delta in chunks:
                for h in range(2):
                    pt = pp.tile([P, 512], f32, tag="ps", name=f"ps_{c}_{h}")
                    nc.tensor.matmul(
                        pt[0:N, 0:HTOK],
                        eblocks[c][0:K, 0:N],
                        X[0:K, c, h * HTOK : (h + 1) * HTOK],
                        start=True,
                        stop=True,
                    )
                    dst = YT[0:N, c, h * HTOK : (h + 1) * HTOK]
                    if has_bias:
                        # fold bias into the PSUM->SBUF cast copy
                        nc.vector.tensor_scalar(
                            dst,
                            pt[0:N, 0:HTOK],
                            bias_sb[0:N, c : c + 1],
                            None,
                            add,
                        )
                    elif ncopy % 2 == 0:
                        nc.vector.tensor_copy(dst, pt[0:N, 0:HTOK])
                    else:
                        nc.scalar.copy(dst, pt[0:N, 0:HTOK])
                    ncopy += 1
                if NPRE + c < NB:  # pipelined late E-build
                    build_e(NPRE + c)
                # merged 4-chunk stores once the last chunk of each group is
                # copied; alternate the ACT/Pool queues (SP stays loads-only)
                if c % 4 == 3:
                    c0 = c - 3
                    dstp = bass.AP(
                        y_d.tensor,
                        126 * c0 * TOK,
                        [[TOK, 126], [126 * TOK, 4], [1, TOK]],
                    )
                    q = nc.scalar if (c0 // 4) % 2 == 0 else nc.gpsimd
                    q.dma_start(out=dstp, in_=YT[0:126, c0 : c0 + 4, :])
            nc.gpsimd.dma_start(out=y_d[4032:4096, :], in_=YT[0:64, 32, :])

    nc.compile()
    return nc


def _gather_bands_pe_u(connections, nearest_neighbors, weight):
    """Row-diagonal bands for the uniform-chunk fp16 PE kernel.

    Same band semantics as _gather_bands_pe (u[i] = factor of
    eff[i, i-1], v[i] = eff[i, i], w[i] = eff[i, i+1]; products on
    device), but packed per _pe_chunks_u's row windows: column d*NB + c
    holds band_d[R_c + p] at partition p.
    """
    chunks = _pe_chunks_u()
    NB = len(chunks)
    z1 = np.zeros(1, np.float32)

    def pack(u, v, w):
        out = np.zeros((P, 3 * NB), np.float32)
        for d, band in enumerate((u, v, w)):
            for c, R, K, C, N, delta in chunks:
                out[:K, d * NB + c] = band[R : R + K]
        return out

    def bands(m, transposed):
        up = np.ascontiguousarray(np.diagonal(m, 1)).astype(np.float32, copy=False)
        mid = np.ascontiguousarray(np.diagonal(m, 0)).astype(np.float32, copy=False)
        dn = np.ascontiguousarray(np.diagonal(m, -1)).astype(np.float32, copy=False)
        if transposed:  # weight[out, in]: need w[i-1,i], w[i,i], w[i+1,i]
            u = np.concatenate([z1, up])  # weight[i-1, i] = diag(w,+1)[i-1]
            w = np.concatenate([dn, z1])  # weight[i+1, i] = diag(w,-1)[i]
        else:  # conn/nn [i, j]: need m[i, i-1], m[i, i], m[i, i+1]
            u = np.concatenate([z1, dn])  # m[i, i-1] = diag(m,-1)[i-1]
            w = np.concatenate([up, z1])  # m[i, i+1] = diag(m,+1)[i]
        return pack(u, mid, w)

    return (
        bands(connections, False),
        bands(nearest_neighbors, False),
        bands(weight, True),
    )


def _gather_bands_pe(connections, nearest_neighbors, weight):
    """Row-diagonal bands for the PE kernel, packed [128, 3*NB].

    u[i] = factor of eff[i, i-1], v[i] = eff[i, i], w[i] = eff[i, i+1]
    (per input matrix; products are computed on device).  Column d*NB + c
    holds band_d[126c + p] at partition p, zero-padded past index 4095.
    """
    NB = len(_pe_chunks())
    z1 = np.zeros(1, np.float32)

    def pack(u, v, w):
        out = np.zeros((P, 3 * NB), np.float32)
        for d, band in enumerate((u, v, w)):
            for c in range(NB):
                lo = 126 * c
                n = min(P, len(band) - lo)
                if n > 0:
                    out[:n, d * NB + c] = band[lo : lo + n]
        return out

    def bands(m, transposed):
        up = np.ascontiguousarray(np.diagonal(m, 1)).astype(np.float32, copy=False)
        mid = np.ascontiguousarray(np.diagonal(m, 0)).astype(np.float32, copy=False)
        dn = np.ascontiguousarray(np.diagonal(m, -1)).astype(np.float32, copy=False)
        if transposed:  # weight[out, in]: need w[i-1,i], w[i,i], w[i+1,i]
            u = np.concatenate([z1, up])  # weight[i-1, i] = diag(w,+1)[i-1]
            w = np.concatenate([dn, z1])  # weight[i+1, i] = diag(w,-1)[i]
        else:  # conn/nn [i, j]: need m[i, i-1], m[i, i], m[i, i+1]
            u = np.concatenate([z1, dn])  # m[i, i-1] = diag(m,-1)[i-1]
            w = np.concatenate([up, z1])  # m[i, i+1] = diag(m,+1)[i]
        return pack(u, mid, w)

    return (
        bands(connections, False),
        bands(nearest_neighbors, False),
        bands(weight, True),
    )


def _gather_bands(connections, nearest_neighbors, weight):
    """Pure indexing: extract the 3 relevant diagonals of each operand.

    Row 0 (A): entries for eff[j-1, j]  -> conn[j-1,j], nn[j-1,j], w[j,j-1]
    Row 1 (B): entries for eff[j, j]    -> conn[j,j],   nn[j,j],   w[j,j]
    Row 2 (C): entries for eff[j+1, j]  -> conn[j+1,j], nn[j+1,j], w[j,j+1]
    Out-of-range slots are zero-padded.
    """
    z1 = np.zeros(1, np.float32)

    def band3(m, transposed):
        # For conn/nn (indexed [i, j] = [row, out-col]):
        #   A[j] = m[j-1, j] = diag(m, +1) shifted;  B = diag(m, 0);
        #   C[j] = m[j+1, j] = diag(m, -1)
        # For weight (indexed [out, in] -> we need w[j, j-1], w[j,j], w[j,j+1]):
        #   A[j] = w[j, j-1] = diag(w, -1) shifted;  B = diag(w, 0);
        #   C[j] = w[j, j+1] = diag(w, +1)
        up = np.ascontiguousarray(np.diagonal(m, 1)).astype(np.float32, copy=False)
        mid = np.ascontiguousarray(np.diagonal(m, 0)).astype(np.float32, copy=False)
        dn = np.ascontiguousarray(np.diagonal(m, -1)).astype(np.float32, copy=False)
        if transposed:  # weight
            a = np.concatenate([z1, dn])
            c = np.concatenate([up, z1])
        else:  # conn / nn
            a = np.concatenate([z1, up])
            c = np.concatenate([dn, z1])
        return np.ascontiguousarray(np.stack([a, mid, c]))

    return (
        band3(connections, False),
        band3(nearest_neighbors, False),
        band3(weight, True),
    )


def kernel(x, connections, nearest_neighbors, weight, bias):
    global LAST_RESULTS
    x = np.asarray(x, dtype=np.float32)
    connections = np.asarray(connections, dtype=np.float32)
    nearest_neighbors = np.asarray(nearest_neighbors, dtype=np.float32)
    weight = np.asarray(weight, dtype=np.float32)
    bias = np.asarray(bias, dtype=np.float32)

    # Safety net: the device kernel assumes nearest_neighbors is zero
    # outside the tridiagonal band (true for this problem by construction).
    i = np.arange(FEAT)
    off_band = np.abs(i[:, None] - i[None, :]) > 1
    if np.any(nearest_neighbors[off_band] != 0.0):
        eff = connections * nearest_neighbors * weight.T
        return (x @ eff + bias).astype(np.float32)

    from concourse.bass_utils import run_bass_kernel_spmd

    has_bias = bool(np.any(bias != 0.0))
    impl = os.environ.get("KERNEL_IMPL", "pe16")
    key = (impl, has_bias)
    if key not in _cached:
        builder = {
            "pe16": _build_banded_pe16_program,
            "pe": _build_banded_pe_program,
            "banded": _build_banded_program,
        }[impl]
        _cached[key] = builder(has_bias)
    nc = _cached[key]

    in_maps = []
    if impl == "pe16":
        cb, nb, wb = _gather_bands_pe_u(connections, nearest_neighbors, weight)
        bands = np.concatenate([cb, nb, wb], axis=1).astype(np.float16)
        xT = x.astype(np.float16).T  # [FEAT, BATCH] view of the fp16 cast
        for c in range(N_CORES):
            m = {
                "xT": np.ascontiguousarray(
                    xT[:, c * TOK_PER_CORE : (c + 1) * TOK_PER_CORE]
                ),
                "bands": bands,
            }
            if has_bias:
                m["bias"] = np.ascontiguousarray(bias.reshape(FEAT, 1))
            in_maps.append(m)
    elif impl == "pe":
        cb, nb, wb = _gather_bands_pe(connections, nearest_neighbors, weight)
        xT = np.ascontiguousarray(x.T)
        for c in range(N_CORES):
            m = {
                "xT": np.ascontiguousarray(
                    xT[:, c * TOK_PER_CORE : (c + 1) * TOK_PER_CORE]
                ),
                "cbT": cb,
                "nbT": nb,
                "wbT": wb,
            }
            if has_bias:
                m["bias"] = np.ascontiguousarray(bias.reshape(1, FEAT))
            in_maps.append(m)
    else:
        cb, nb, wb = _gather_bands(connections, nearest_neighbors, weight)
        for c in range(N_CORES):
            m = {
                "x": np.ascontiguousarray(
                    x[c * TOK_PER_CORE : (c + 1) * TOK_PER_CORE, :]
                ),
                "conn_band": cb,
                "nn_band": nb,
                "w_band": wb,
            }
            if has_bias:
                m["bias"] = np.ascontiguousarray(bias.reshape(1, FEAT))
            in_maps.append(m)

    trace = bool(int(os.environ.get("KERNEL_TRACE", "0")))
    res = run_bass_kernel_spmd(
        nc, in_maps, core_ids=list(range(N_CORES)), trace=trace
    )
    LAST_RESULTS = res

    out = np.empty((BATCH, FEAT), dtype=np.float32)
    for c in range(N_CORES):
        yc = np.asarray(res.results[c]["y"])
        if yc.shape == (FEAT, TOK_PER_CORE):  # transposed (pe16) layout
            yc = yc.T
        out[c * TOK_PER_CORE : (c + 1) * TOK_PER_CORE, :] = yc.astype(
            np.float32
        )
    return out

